# revision 1
# baseline (speedup 1.0000x reference)
"""TRN2 Bass kernel for nn_CIFAR10_Type1_Template_Unroll (dense_cnn).

Network (per reference): two locally-connected conv layers + 3-layer FC
head, B=4096, all fp32. Strategy: pure data parallel over 8 NeuronCores
(512 batch each), activations kept on-chip in [feature, batch] layout.
Matmuls run in fp32r (full PE rate for N>=256, ~1e-4 relative error)
except L2, which runs in fp16 (~5e-4) so pairs of output positions can
execute concurrently on the PE's column strips (tile_position col
tiling is rejected by walrus for 4-byte dtypes).

Layer mapping per core (batch N=512 on the matmul free dim throughout):
- L1 (k=2,s=2 locally-connected): patches are a pure reshape of x. Host
  packs, per output row r and pair of adjacent positions, a K=32 strip
  (2 positions x 16 feats: 12 real + 4 zero-pad) and a block-diagonal
  [32, 128] weight tile. 4 pairs run concurrently on the PE's 32-row
  strips via tile_position=(32i, 0).
- L2 (k=4,s=2): positions are paired (matching the h2 tile halves);
  the two members run concurrently on col strips 0-63 / 64-127 via
  tile_position (0,0)/(0,64), each accumulating 8 K-tile matmuls
  [128K, 64M] in its own PSUM bank (start=True clears a whole bank, so
  chains must not share one).
- FC head: standard K/M tiling; FC weights host-permuted to match the
  on-chip feature order of h2 ([pos-pair, parity, channel]).
Host-side prep only reshapes/permutes weights and input (numpy).
"""
import sys

if '/opt/trn_rl_repo' not in sys.path:
    sys.path.insert(0, '/opt/trn_rl_repo')

import numpy as np

N_CORES = 8
BS = 512
LAST_EXEC_NS = None

# ----------------------------------------------------------------- host prep

def _prep_x(x):
    """x [B,3,32,32] -> [N_CORES, 16, 2, 128, 512] patch tiles.

    part = 32*i + 16*q + f; pair p=4g+i covers w1 in {2p,2p+1}; q = w1
    parity; f = c*4 + kh*2 + kw (12..15 zero-pad). Free dim = batch.
    """
    ncr = x.shape[0] // BS
    xr = x.reshape(ncr, BS, 3, 16, 2, 2, 4, 2, 2)   # s,b,c,r,kh,g,i,q,kw
    xt = xr.transpose(0, 3, 5, 6, 7, 2, 4, 8, 1)    # s,r,g,i,q,c,kh,kw,b
    xt = xt.reshape(ncr, 16, 2, 4, 2, 12, BS)
    xpp = np.zeros((ncr, 16, 2, 4, 2, 16, BS), np.float32)
    xpp[..., :12, :] = xt
    return np.ascontiguousarray(xpp.reshape(ncr, 16, 2, 128, BS))


def _prep_w1(conv1w):
    """conv1w [64,256,3,2,2] -> [16, 128, 2, 128] block-diag strips."""
    w1r = conv1w.reshape(64, 16, 16, 3, 2, 2)
    wt = w1r.transpose(1, 2, 3, 4, 5, 0).reshape(16, 16, 12, 64)
    wtp = np.zeros((16, 16, 16, 64), np.float32)
    wtp[:, :, :12, :] = wt
    wtp = wtp.reshape(16, 2, 4, 2, 16, 64)          # r,g,i,qp,f,o
    w1t = np.zeros((16, 2, 4, 2, 16, 2, 64), np.float32)
    w1t[:, :, :, 0, :, 0, :] = wtp[:, :, :, 0, :, :]
    w1t[:, :, :, 1, :, 1, :] = wtp[:, :, :, 1, :, :]
    w1t = w1t.reshape(16, 2, 128, 128).transpose(0, 2, 1, 3)
    return np.ascontiguousarray(w1t)


def _prep_w2(conv2w):
    """conv2w [64,49,64,4,4] -> [49, 128, 512] (fp16)."""
    w2r = conv2w.reshape(64, 7, 7, 64, 4, 4)
    v = w2r.transpose(1, 2, 3, 4, 5, 0)             # h,w,c,kh,kw,o
    v = v.reshape(7, 7, 64, 4, 2, 2, 64)            # h,w,c,kh,t,q,o
    v = v.transpose(0, 1, 5, 2, 3, 4, 6)            # h,w,q,c,kh,t,o
    return np.ascontiguousarray(v.reshape(49, 128, 512)).astype(np.float16)


def _h2_posmap():
    pm = np.full((25, 2), -1, np.int64)
    for T in range(21):
        rr, j = divmod(T, 3)
        pm[T, 0] = rr * 7 + 2 * j
        pm[T, 1] = rr * 7 + 2 * j + 1
    for pi in range(4):
        r0, r1 = 2 * pi, 2 * pi + 1
        pm[21 + pi, 0] = r0 * 7 + 6
        if r1 < 7:
            pm[21 + pi, 1] = r1 * 7 + 6
    return pm


def _prep_fc1(fc1):
    pm = _h2_posmap()
    fc1p = fc1.reshape(1024, 64, 49)
    fc1hat = np.zeros((1024, 25, 2, 64), np.float32)
    for T in range(25):
        for u in range(2):
            p = pm[T, u]
            if p >= 0:
                fc1hat[:, T, u, :] = fc1p[:, :, p]
    a = fc1hat.reshape(1024, 25, 128).reshape(8, 128, 25, 128)
    return np.ascontiguousarray(a.transpose(0, 3, 2, 1))   # m,kp,k,mc


def _prep_fc2(fc2):
    a = fc2.reshape(4, 128, 8, 128)
    return np.ascontiguousarray(a.transpose(0, 3, 2, 1))   # m,kp,k,mc


def _prep_fc3(fc3):
    a = fc3.T.reshape(4, 128, 10)
    return np.ascontiguousarray(a.transpose(1, 0, 2))      # kp,k,o


# --------------------------------------------------------------- bass kernel

_NC_CACHE = []


def _build_nc():
    import concourse.bass as bass
    import concourse.mybir as mybir
    from concourse import bacc
    from concourse.tile import TileContext

    f32 = mybir.dt.float32
    f32r = mybir.dt.float32r
    f16 = mybir.dt.float16
    RELU = mybir.ActivationFunctionType.Relu
    rc = lambda ap: ap

    nc = bacc.Bacc("TRN2", target_bir_lowering=False, debug=False,
                   num_devices=N_CORES)
    x_pp = nc.dram_tensor("x_pp", [16, 2, 128, BS], f32r, kind="ExternalInput")
    w1t = nc.dram_tensor("w1t", [16, 128, 2, 128], f32r, kind="ExternalInput")
    w2t = nc.dram_tensor("w2t", [49, 128, 512], f16, kind="ExternalInput")
    fc1m = nc.dram_tensor("fc1m", [8, 128, 25, 128], f32r, kind="ExternalInput")
    fc2t = nc.dram_tensor("fc2t", [4, 128, 8, 128], f32r, kind="ExternalInput")
    fc3t = nc.dram_tensor("fc3t", [128, 4, 10], f32r, kind="ExternalInput")
    zeros64 = nc.dram_tensor("zeros64", [64, 512], f32r, kind="ExternalInput")
    y = nc.dram_tensor("y", [BS, 10], f32, kind="ExternalOutput")

    pm = _h2_posmap()
    tile_of_pos = {}
    for T in range(25):
        for u in range(2):
            if pm[T, u] >= 0:
                tile_of_pos[pm[T, u]] = (T, u)

    ectr = [0]

    with TileContext(nc) as tc:
        def relu_evac(dst, src):
            if ectr[0] % 2 == 0:
                nc.scalar.activation(dst, src, RELU)
            else:
                nc.vector.tensor_scalar_max(dst, src, 0.0)
            ectr[0] += 1

        with (
            tc.tile_pool(name="h2pool", bufs=25) as h2pool,
            tc.tile_pool(name="fcw", bufs=2) as fcw_pool,
        ):
            h2 = [h2pool.tile([128, 512], f32r, tag="h2", name=f"h2_{T}")
                  for T in range(25)]
            # --------------- phase 1: L1 + L2 interleaved ---------------
            with (
                tc.tile_pool(name="xp", bufs=4) as xpp_pool,
                tc.tile_pool(name="w1p", bufs=3) as w1_pool,
                tc.tile_pool(name="w2p", bufs=6) as w2_pool,
                tc.tile_pool(name="o1p", bufs=72) as o1_pool,
                tc.tile_pool(name="l1ps", bufs=4, space="PSUM") as l1ps,
                tc.tile_pool(name="l2ps", bufs=4, space="PSUM") as l2ps,
            ):
                nc.sync.dma_start(out=h2[24][64:128, :], in_=zeros64.ap()[:])
                # PE warmup: keep the array busy during the initial DMA
                # ramp so HAM un-throttles before real matmuls arrive.
                # Dummy MMs over the (already zeroed) h2[24] hi half; the
                # consumed psum bank is start=True-cleared by later users.
                wps = l1ps.tile([128, 512], f32, tag="l1", name="warm_ps")
                zsrc = h2[24][64:96, 0:512]
                for wi in range(14):
                    nc.tensor.matmul(wps[:], zsrc[:, 0:128], zsrc[:, :],
                                     start=True, stop=True)
                out1 = [[None] * 8 for _ in range(16)]

                def emit_l1_row(r):
                    w1row = w1_pool.tile([128, 256], f32r, tag="w1",
                                         name=f"w1_{r}")
                    w1src = w1t.ap()[r].rearrange("p g c -> p (g c)")
                    if r == 0:
                        for i in range(4):
                            nc.sync.dma_start(out=w1row[32*i:32*i+32, :],
                                              in_=w1src[32*i:32*i+32, :])
                    else:
                        nc.sync.dma_start(out=w1row[:], in_=w1src)
                    for g in range(2):
                        xt = xpp_pool.tile([128, BS], f32r, tag="xp",
                                           name=f"xp_{r}_{g}")
                        if r == 0:
                            for i in range(4):
                                nc.sync.dma_start(
                                    out=xt[32*i:32*i+32, :],
                                    in_=x_pp.ap()[r, g][32*i:32*i+32, :])
                        else:
                            nc.sync.dma_start(out=xt[:], in_=x_pp.ap()[r, g])
                        for i in range(4):
                            ps = l1ps.tile([128, 512], f32, tag="l1",
                                           name=f"l1ps_{r}_{g}_{i}")
                            nc.tensor.matmul(
                                ps[:],
                                rc(w1row[32*i:32*i+32, 128*g:128*g+128]),
                                rc(xt[32*i:32*i+32, :]),
                                start=True, stop=True,
                                tile_position=(32 * i, 0))
                            ot = o1_pool.tile([128, 512], f16, tag="o1",
                                              name=f"o1_{r}_{4*g+i}")
                            relu_evac(ot[:], ps[:])
                            out1[r][4 * g + i] = ot

                def load_w2(pos):
                    w2til = w2_pool.tile([128, 512], f16, tag="w2",
                                         name=f"w2_{pos}")
                    nc.sync.dma_start(out=w2til[:], in_=w2t.ap()[pos])
                    return w2til

                def emit_l2_pair(T, hA, wA, hB, wB):
                    # Two positions concurrently on PE col strips 0-63 /
                    # 64-127 (tile_position col tiling), each chain
                    # accumulating in its own PSUM bank so the start=True
                    # bank clears stay independent of scheduler order.
                    wtA = load_w2(hA * 7 + wA)
                    wtB = None if hB is None else load_w2(hB * 7 + wB)
                    psA = l2ps.tile([128, 512], f32, tag="l2",
                                    name=f"l2psA_{T}")
                    psB = None
                    if wtB is not None:
                        psB = l2ps.tile([128, 512], f32, tag="l2",
                                        name=f"l2psB_{T}")
                    for kt in range(8):
                        kh, t = divmod(kt, 2)
                        nc.tensor.matmul(
                            psA[0:64, :],
                            wtA[:, 64*kt:64*kt+64],
                            out1[2*hA+kh][wA+t][:],
                            start=(kt == 0), stop=(kt == 7),
                            tile_position=(0, 0))
                        if wtB is not None:
                            nc.tensor.matmul(
                                psB[64:128, :],
                                wtB[:, 64*kt:64*kt+64],
                                out1[2*hB+kh][wB+t][:],
                                start=(kt == 0), stop=(kt == 7),
                                tile_position=(0, 64))
                    relu_evac(h2[T][0:64, :], psA[0:64, :])
                    if wtB is not None:
                        relu_evac(h2[T][64:128, :], psB[64:128, :])

                def emit_l2_pass(h):
                    for j in range(3):
                        emit_l2_pair(h * 3 + j, h, 2 * j, h, 2 * j + 1)
                    # cross pairs (w=6, rows h-2 & h-1) are deferred one
                    # pass: their hi-chain rhs tiles are the last evacs of
                    # row 2h-1, and the in-order PE would stall
                    # head-of-line waiting for them if emitted in pass h-1.
                    if h >= 2 and h % 2 == 0:
                        pi = (h - 2) // 2
                        emit_l2_pair(21 + pi, h - 2, 6, h - 1, 6)
                    if h == 6:
                        # Re-emitting T=23 here is intentional: it writes
                        # identical data a second time, but the extra pair
                        # keeps the PE stream dense across the last L2 pass
                        # and measures consistently faster.
                        emit_l2_pair(23, 4, 6, 5, 6)
                        emit_l2_pair(24, 6, 6, None, None)

                for r in range(16):
                    emit_l1_row(r)
                    if r == 1:
                        # second keep-warm burst: l2ps banks are idle until
                        # the first L2 pass; fills the DMA-paced early rows
                        # so HAM stays un-throttled.
                        wps2 = l2ps.tile([128, 512], f32, tag="l2",
                                         name="warm_ps2")
                        for wi in range(10):
                            nc.tensor.matmul(wps2[:], zsrc[:, 0:128],
                                             zsrc[:, :],
                                             start=True, stop=True)
                    if r >= 3 and r % 2 == 1:
                        emit_l2_pass((r - 3) // 2)

            # --------------- phase 2: FC head ---------------
            with (
                tc.tile_pool(name="fcio", bufs=12) as fcio_pool,
                tc.tile_pool(name="fcps", bufs=2, space="PSUM") as fcps,
                tc.tile_pool(name="fc3ps", bufs=2, space="PSUM") as fc3ps,
            ):
                h3 = []
                for m in range(8):
                    wt = fcw_pool.tile([128, 25 * 128], f32r, tag="fc1w",
                                       name=f"fc1w_{m}")
                    src = fc1m.ap()[m].rearrange("p k c -> p (k c)")
                    nc.sync.dma_start(out=wt[:, 0:1600], in_=src[:, 0:1600])
                    nc.sync.dma_start(out=wt[:, 1600:3200],
                                      in_=src[:, 1600:3200])
                    ps = fcps.tile([128, 512], f32, tag="fc",
                                   name=f"fc1ps_{m}")
                    for k in range(25):
                        nc.tensor.matmul(ps[:],
                                         rc(wt[:, 128*k:128*k+128]),
                                         rc(h2[k][:]),
                                         start=(k == 0), stop=(k == 24))
                    ot = fcio_pool.tile([128, 512], f32r, tag="h3",
                                        name=f"h3_{m}", bufs=8)
                    relu_evac(ot[:], ps[:])
                    h3.append(ot)
                h4 = []
                for m in range(4):
                    wt = fcw_pool.tile([128, 8 * 128], f32r, tag="fc2w",
                                       name=f"fc2w_{m}")
                    nc.sync.dma_start(
                        out=wt[:],
                        in_=fc2t.ap()[m].rearrange("p k c -> p (k c)"))
                    ps = fcps.tile([128, 512], f32, tag="fc",
                                   name=f"fc2ps_{m}")
                    for k in range(8):
                        nc.tensor.matmul(ps[:],
                                         rc(wt[:, 128*k:128*k+128]),
                                         rc(h3[k][:]),
                                         start=(k == 0), stop=(k == 7))
                    ot = fcio_pool.tile([128, 512], f32r, tag="h4",
                                        name=f"h4_{m}", bufs=4)
                    relu_evac(ot[:], ps[:])
                    h4.append(ot)
                w3 = fcio_pool.tile([128, 40], f32r, tag="fc3w",
                                    name="fc3w", bufs=1)
                nc.sync.dma_start(
                    out=w3[:], in_=fc3t.ap().rearrange("p k o -> p (k o)"))
                for b4 in range(4):
                    ps = fc3ps.tile([128, 10], f32, tag="fc3",
                                    name=f"fc3ps_{b4}")
                    for k in range(4):
                        nc.tensor.matmul(
                            ps[:],
                            rc(h4[k][:, 128*b4:128*b4+128]),
                            rc(w3[:, 10*k:10*k+10]),
                            start=(k == 0), stop=(k == 3))
                    ot = fcio_pool.tile([128, 10], f32, tag="yout",
                                        name=f"y_{b4}", bufs=4)
                    nc.vector.tensor_copy(ot[:], ps[:])
                    nc.sync.dma_start(out=y.ap()[128*b4:128*b4+128, :],
                                      in_=ot[:])
    nc.compile()
    return nc


def kernel(x, conv1w, conv2w, fc1, fc2, fc3):
    global LAST_EXEC_NS
    from concourse.bass_utils import run_bass_kernel_spmd

    x = np.ascontiguousarray(np.asarray(x, dtype=np.float32))
    conv1w = np.ascontiguousarray(np.asarray(conv1w, dtype=np.float32))
    conv2w = np.ascontiguousarray(np.asarray(conv2w, dtype=np.float32))
    fc1 = np.ascontiguousarray(np.asarray(fc1, dtype=np.float32))
    fc2 = np.ascontiguousarray(np.asarray(fc2, dtype=np.float32))
    fc3 = np.ascontiguousarray(np.asarray(fc3, dtype=np.float32))

    if not _NC_CACHE:
        _NC_CACHE.append(_build_nc())
    nc = _NC_CACHE[0]

    xpp = _prep_x(x)
    shared = {
        "zeros64": np.zeros((64, 512), np.float32),
        "w1t": _prep_w1(conv1w),
        "w2t": _prep_w2(conv2w),
        "fc1m": _prep_fc1(fc1),
        "fc2t": _prep_fc2(fc2),
        "fc3t": _prep_fc3(fc3),
    }
    in_maps = [{**shared, "x_pp": xpp[c]} for c in range(N_CORES)]
    res = run_bass_kernel_spmd(nc, in_maps, list(range(N_CORES)))
    LAST_EXEC_NS = res.exec_time_ns
    return np.concatenate([r["y"] for r in res.results], axis=0)



# revision 3
# speedup vs baseline: 1.1658x; 1.1658x over previous
"""TRN2 Bass kernel for nn_CIFAR10_Type1_Template_Unroll (dense_cnn).

Network (per reference): two locally-connected conv layers + 3-layer FC
head, B=4096, fp32 in/out. Pure data parallel over 8 NeuronCores (512
batch each), activations on-chip in [feature, batch] layout, batch N=512
on the matmul free dim throughout. All DMA'd operands are fp16 (inputs
are O(1) normals; rounding is ~5e-4 relative, budget is 2e-2), halving
HBM traffic vs fp32 so phase 1 stays PE-bound instead of DMA-bound.

Layer mapping per core:
- L1 (k=2,s=2 locally-connected): host packs, per output row r and pair
  of adjacent positions, a K=32 strip (2 positions x 16 feats: 12 real +
  4 zero-pad) and a block-diagonal [32, 128] weight tile. 4 strips run
  concurrently on the PE's 32-row groups via tile_position=(32i, 0),
  writing two 2-bank PSUM doubles that are evacuated with single
  [128,1024] relu ops (evac cost scales with free dim only, so merging
  banks halves the fixed+per-bank cost).
- L2 (k=4,s=2): positions are paired (h,2j)+(h,2j+1). The two positions
  share the middle input pair (2j+1), so a host-packed block weight
  [128,128] lets ONE full-width matmul per kh start the accumulation
  group for BOTH positions in one PSUM bank; the outer input pairs run
  as M=64 chains on PE column strips (0,0)/(0,64). One [128,512] relu
  evac per pair instead of two [64,512] halves. Column-7 positions pair
  across rows on column strips with separate banks (no shared rhs).
- FC head: weights prefetched into SBUF during phase 1 (fp16: fc1 is
  6.6 MB), K/M tiling, fc weights host-permuted to match the on-chip
  feature order of h2 ([pos-pair, parity, channel]).
A 12-matmul warmup on a memset scratch tile (no DMA dependency) ramps
the PE HAM clock gate from t~0 and bridges the first x-tile DMA.
"""
import sys

if '/opt/trn_rl_repo' not in sys.path:
    sys.path.insert(0, '/opt/trn_rl_repo')

import numpy as np

N_CORES = 8
BS = 512
LAST_EXEC_NS = None

# ----------------------------------------------------------------- host prep

def _prep_x(x):
    """x [B,3,32,32] -> [N_CORES, 16, 2, 128, 512] fp16 patch tiles.

    part = 32*i + 16*q + f; pair p=4g+i covers w1 in {2p,2p+1}; q = w1
    parity; f = c*4 + kh*2 + kw (12..15 zero-pad). Free dim = batch.
    """
    ncr = x.shape[0] // BS
    xr = x.reshape(ncr, BS, 3, 16, 2, 2, 4, 2, 2)   # s,b,c,r,kh,g,i,q,kw
    xt = xr.transpose(0, 3, 5, 6, 7, 2, 4, 8, 1)    # s,r,g,i,q,c,kh,kw,b
    xt = xt.reshape(ncr, 16, 2, 4, 2, 12, BS)
    xpp = np.zeros((ncr, 16, 2, 4, 2, 16, BS), np.float16)
    xpp[..., :12, :] = xt
    return np.ascontiguousarray(xpp.reshape(ncr, 16, 2, 128, BS))


def _prep_w1(conv1w):
    """conv1w [64,256,3,2,2] -> [16, 128, 2, 128] fp16 block-diag strips."""
    w1r = conv1w.reshape(64, 16, 16, 3, 2, 2)
    wt = w1r.transpose(1, 2, 3, 4, 5, 0).reshape(16, 16, 12, 64)
    wtp = np.zeros((16, 16, 16, 64), np.float32)
    wtp[:, :, :12, :] = wt
    wtp = wtp.reshape(16, 2, 4, 2, 16, 64)          # r,g,i,qp,f,o
    w1t = np.zeros((16, 2, 4, 2, 16, 2, 64), np.float32)
    w1t[:, :, :, 0, :, 0, :] = wtp[:, :, :, 0, :, :]
    w1t[:, :, :, 1, :, 1, :] = wtp[:, :, :, 1, :, :]
    w1t = w1t.reshape(16, 2, 128, 128).transpose(0, 2, 1, 3)
    return np.ascontiguousarray(w1t).astype(np.float16)


def _prep_w2(conv2w):
    """conv2w [64,49,64,4,4] -> main-pair [21,128,1024] + col-6 [7,128,512].

    Main pair T=(h, w'=2j / 2j+1): both positions read the middle input
    pair 2j+1, so cols 0:512 hold, per kh, the [128,128] combined block
    (A's t=1 weights | B's t=0 weights); cols 512:768 = A's left-pair
    (t=0) M=64 blocks; cols 768:1024 = B's right-pair (t=1) blocks.
    Partition = (q=kw%2)*64 + c, matching the L1 output tile layout.
    """
    wr = conv2w.reshape(64, 7, 7, 64, 4, 2, 2)      # o,h,w',c,kh,t,q
    wA = wr[:, :, 0:6:2]                            # o,h,j,c,kh,t,q
    wB = wr[:, :, 1:6:2]
    # [h,j,q,c,kh,o] each
    amid = wA[:, :, :, :, :, 1, :].transpose(1, 2, 5, 3, 4, 0)
    aleft = wA[:, :, :, :, :, 0, :].transpose(1, 2, 5, 3, 4, 0)
    bmid = wB[:, :, :, :, :, 0, :].transpose(1, 2, 5, 3, 4, 0)
    bright = wB[:, :, :, :, :, 1, :].transpose(1, 2, 5, 3, 4, 0)
    comb = np.stack([amid, bmid], axis=5)           # h,j,q,c,kh,half,o
    comb = comb.reshape(7, 3, 2, 64, 512)
    w2p = np.concatenate(
        [comb, aleft.reshape(7, 3, 2, 64, 256),
         bright.reshape(7, 3, 2, 64, 256)], axis=-1)
    w2p = np.ascontiguousarray(w2p.reshape(21, 128, 1024)).astype(np.float16)
    # column-6 positions, per-position layout: cols = (kh, t, o)
    w6 = wr[:, :, 6]                                # o,h,c,kh,t,q
    w2s = w6.transpose(1, 5, 2, 3, 4, 0).reshape(7, 128, 512)
    return w2p, np.ascontiguousarray(w2s).astype(np.float16)


def _h2_posmap():
    pm = np.full((25, 2), -1, np.int64)
    for T in range(21):
        rr, j = divmod(T, 3)
        pm[T, 0] = rr * 7 + 2 * j
        pm[T, 1] = rr * 7 + 2 * j + 1
    for pi in range(4):
        r0, r1 = 2 * pi, 2 * pi + 1
        pm[21 + pi, 0] = r0 * 7 + 6
        if r1 < 7:
            pm[21 + pi, 1] = r1 * 7 + 6
    return pm


def _prep_fc1(fc1):
    pm = _h2_posmap()
    fc1p = fc1.reshape(1024, 64, 49)
    fc1hat = np.zeros((1024, 25, 2, 64), np.float32)
    for T in range(25):
        for u in range(2):
            p = pm[T, u]
            if p >= 0:
                fc1hat[:, T, u, :] = fc1p[:, :, p]
    a = fc1hat.reshape(1024, 25, 128).reshape(8, 128, 25, 128)
    return np.ascontiguousarray(a.transpose(0, 3, 2, 1)).astype(np.float16)


def _prep_fc2(fc2):
    a = fc2.reshape(4, 128, 8, 128)
    return np.ascontiguousarray(a.transpose(0, 3, 2, 1)).astype(np.float16)


def _prep_fc3(fc3):
    a = fc3.T.reshape(4, 128, 10).transpose(1, 0, 2)
    return np.ascontiguousarray(a.reshape(128, 40)).astype(np.float16)


# --------------------------------------------------------------- bass kernel

_NC_CACHE = []


def _build_nc():
    import concourse.bass as bass
    import concourse.mybir as mybir
    from concourse import bacc
    from concourse.tile import TileContext

    f32 = mybir.dt.float32
    f16 = mybir.dt.float16
    RELU = mybir.ActivationFunctionType.Relu

    nc = bacc.Bacc("TRN2", target_bir_lowering=False, debug=False,
                   num_devices=N_CORES)
    x_pp = nc.dram_tensor("x_pp", [16, 2, 128, BS], f16, kind="ExternalInput")
    w1t = nc.dram_tensor("w1t", [16, 128, 2, 128], f16, kind="ExternalInput")
    w2p = nc.dram_tensor("w2p", [21, 128, 1024], f16, kind="ExternalInput")
    w2s = nc.dram_tensor("w2s", [7, 128, 512], f16, kind="ExternalInput")
    fc1m = nc.dram_tensor("fc1m", [8, 128, 25, 128], f16, kind="ExternalInput")
    fc2t = nc.dram_tensor("fc2t", [4, 128, 8, 128], f16, kind="ExternalInput")
    fc3t = nc.dram_tensor("fc3t", [128, 40], f16, kind="ExternalInput")
    y = nc.dram_tensor("y", [BS, 10], f32, kind="ExternalOutput")

    ectr = [0]

    with TileContext(nc) as tc:
        def relu_evac(dst, src):
            if ectr[0] % 2 == 0:
                nc.scalar.activation(dst, src, RELU)
            else:
                nc.vector.tensor_scalar_max(dst, src, 0.0)
            ectr[0] += 1

        with (
            tc.tile_pool(name="h2pool", bufs=25) as h2pool,
            tc.tile_pool(name="fcw", bufs=8) as fcw_pool,
            tc.tile_pool(name="scratch", bufs=1) as sc_pool,
        ):
            h2 = [h2pool.tile([128, 512], f16, tag="h2", name=f"h2_{T}")
                  for T in range(25)]
            ws = sc_pool.tile([32, 512], f16, tag="ws", name="ws")
            # fc1 weight tiles prefetched chunk-wise through phase 1
            fc1w = [fcw_pool.tile([128, 25 * 128], f16, tag="fc1w",
                                  name=f"fc1w_{m}") for m in range(8)]
            # --------------- phase 1: L1 + L2 interleaved ---------------
            with (
                tc.tile_pool(name="xp", bufs=6) as xpp_pool,
                tc.tile_pool(name="w1p", bufs=3) as w1_pool,
                tc.tile_pool(name="w2pp", bufs=4) as w2p_pool,
                tc.tile_pool(name="w2sp", bufs=3) as w2s_pool,
                tc.tile_pool(name="o1p", bufs=36) as o1_pool,
                tc.tile_pool(name="l1ps", bufs=2, space="PSUM") as l1ps,
                tc.tile_pool(name="l2ps", bufs=4, space="PSUM") as l2ps,
            ):
                # PE warmup with no DMA dependency: memset scratch, then
                # dummy MMs so HAM un-throttles while the first x tiles
                # stream in.
                nc.vector.memset(ws[:], 0.0)
                nc.vector.memset(h2[24][64:128, :], 0.0)
                wps = l2ps.tile([128, 512], f32, tag="l2", name="warm_ps")
                for wi in range(12):
                    nc.tensor.matmul(wps[:], ws[:, 0:128], ws[:],
                                     start=True, stop=True,
                                     tile_position=(0, 0))

                out1d = [[None] * 4 for _ in range(16)]

                def out1(r, p):
                    u = p % 2
                    return out1d[r][p // 2][:, 512 * u:512 * u + 512]

                def emit_l1_row(r):
                    w1row = w1_pool.tile([128, 256], f16, tag="w1",
                                         name=f"w1_{r}")
                    w1src = w1t.ap()[r].rearrange("p g c -> p (g c)")
                    if r == 0:
                        for i in range(4):
                            nc.sync.dma_start(out=w1row[32*i:32*i+32, :],
                                              in_=w1src[32*i:32*i+32, :])
                    else:
                        nc.sync.dma_start(out=w1row[:], in_=w1src)
                    for g in range(2):
                        xt = xpp_pool.tile([128, BS], f16, tag="xp",
                                           name=f"xp_{r}_{g}")
                        if r == 0:
                            for i in range(4):
                                nc.sync.dma_start(
                                    out=xt[32*i:32*i+32, :],
                                    in_=x_pp.ap()[r, g][32*i:32*i+32, :])
                        else:
                            nc.sync.dma_start(out=xt[:], in_=x_pp.ap()[r, g])
                        pss = []
                        for d in range(2):
                            ps = l1ps.tile([128, 1024], f32, tag="l1",
                                           name=f"l1ps_{r}_{g}_{d}")
                            pss.append(ps)
                            for u in range(2):
                                i = 2 * d + u
                                nc.tensor.matmul(
                                    ps[:, 512*u:512*u+512],
                                    w1row[32*i:32*i+32, 128*g:128*g+128],
                                    xt[32*i:32*i+32, :],
                                    start=True, stop=True,
                                    tile_position=(32 * i, 0))
                        for d in range(2):
                            ot = o1_pool.tile([128, 1024], f16, tag="o1",
                                              name=f"o1_{r}_{g}_{d}")
                            relu_evac(ot[:], pss[d][:])
                            out1d[r][2 * g + d] = ot

                def emit_main_pair(h, j):
                    T = 3 * h + j
                    wt = w2p_pool.tile([128, 1024], f16, tag="w2p",
                                       name=f"w2p_{T}")
                    nc.sync.dma_start(out=wt[:], in_=w2p.ap()[T])
                    ps = l2ps.tile([128, 512], f32, tag="l2",
                                   name=f"l2ps_{T}")
                    # combined full-width chain on the shared middle pair
                    for kh in range(4):
                        nc.tensor.matmul(
                            ps[:], wt[:, 128*kh:128*kh+128],
                            out1(2*h + kh, 2*j + 1),
                            start=(kh == 0), stop=False,
                            tile_position=(0, 0))
                    # outer pairs: A (left) on cols 0-63, B (right) on
                    # cols 64-127, interleaved for column-strip overlap
                    for kh in range(4):
                        nc.tensor.matmul(
                            ps[0:64, :], wt[:, 512+64*kh:512+64*kh+64],
                            out1(2*h + kh, 2*j),
                            start=False, stop=(kh == 3),
                            tile_position=(0, 0))
                        nc.tensor.matmul(
                            ps[64:128, :], wt[:, 768+64*kh:768+64*kh+64],
                            out1(2*h + kh, 2*j + 2),
                            start=False, stop=(kh == 3),
                            tile_position=(0, 64))
                    relu_evac(h2[T][:], ps[:])

                def load_w2s(pos_h):
                    wtil = w2s_pool.tile([128, 512], f16, tag="w2s",
                                         name=f"w2s_{pos_h}")
                    nc.sync.dma_start(out=wtil[:], in_=w2s.ap()[pos_h])
                    return wtil

                def emit_cross_pair(pi):
                    # positions (2pi, 6) and (2pi+1, 6) on column strips,
                    # separate PSUM banks (no shared rhs tile)
                    T = 21 + pi
                    hA, hB = 2 * pi, 2 * pi + 1
                    wtA = load_w2s(hA)
                    wtB = load_w2s(hB) if hB < 7 else None
                    psA = l2ps.tile([128, 512], f32, tag="l2",
                                    name=f"l2psA_{T}")
                    psB = None
                    if wtB is not None:
                        psB = l2ps.tile([128, 512], f32, tag="l2",
                                        name=f"l2psB_{T}")
                    for kt in range(8):
                        kh, t = divmod(kt, 2)
                        nc.tensor.matmul(
                            psA[0:64, :], wtA[:, 64*kt:64*kt+64],
                            out1(2*hA + kh, 6 + t),
                            start=(kt == 0), stop=(kt == 7),
                            tile_position=(0, 0))
                        if wtB is not None:
                            nc.tensor.matmul(
                                psB[64:128, :], wtB[:, 64*kt:64*kt+64],
                                out1(2*hB + kh, 6 + t),
                                start=(kt == 0), stop=(kt == 7),
                                tile_position=(0, 64))
                    relu_evac(h2[T][0:64, :], psA[0:64, :])
                    if wtB is not None:
                        relu_evac(h2[T][64:128, :], psB[64:128, :])

                def emit_pass(h):
                    for j in range(3):
                        emit_main_pair(h, j)
                    if h in (1, 3, 5):
                        emit_cross_pair((h - 1) // 2)
                    if h == 6:
                        emit_cross_pair(3)

                for r in range(16):
                    emit_l1_row(r)
                    # fc1 prefetch: one 400 KB chunk per row
                    m, half = r // 2, r % 2
                    src = fc1m.ap()[m].rearrange("p k c -> p (k c)")
                    nc.sync.dma_start(
                        out=fc1w[m][:, 1600*half:1600*half+1600],
                        in_=src[:, 1600*half:1600*half+1600])
                    if r >= 4 and r % 2 == 0:
                        emit_pass((r - 4) // 2)
                emit_pass(6)

            # --------------- phase 2: FC head ---------------
            with (
                tc.tile_pool(name="fcio", bufs=12) as fcio_pool,
                tc.tile_pool(name="fcps", bufs=2, space="PSUM") as fcps,
                tc.tile_pool(name="fc3ps", bufs=2, space="PSUM") as fc3ps,
            ):
                fc2w = [fcw_pool.tile([128, 8 * 128], f16, tag="fc2w",
                                      name=f"fc2w_{m}", bufs=4)
                        for m in range(4)]
                for m in range(4):
                    nc.sync.dma_start(
                        out=fc2w[m][:],
                        in_=fc2t.ap()[m].rearrange("p k c -> p (k c)"))
                w3 = fcw_pool.tile([128, 40], f16, tag="fc3w",
                                   name="fc3w", bufs=1)
                nc.sync.dma_start(out=w3[:], in_=fc3t.ap())
                h3 = []
                for m in range(8):
                    ps = fcps.tile([128, 512], f32, tag="fc",
                                   name=f"fc1ps_{m}")
                    for k in range(25):
                        nc.tensor.matmul(ps[:],
                                         fc1w[m][:, 128*k:128*k+128],
                                         h2[k][:],
                                         start=(k == 0), stop=(k == 24))
                    ot = fcio_pool.tile([128, 512], f16, tag="h3",
                                        name=f"h3_{m}", bufs=8)
                    relu_evac(ot[:], ps[:])
                    h3.append(ot)
                h4 = []
                for m in range(4):
                    ps = fcps.tile([128, 512], f32, tag="fc",
                                   name=f"fc2ps_{m}")
                    for k in range(8):
                        nc.tensor.matmul(ps[:],
                                         fc2w[m][:, 128*k:128*k+128],
                                         h3[k][:],
                                         start=(k == 0), stop=(k == 7))
                    ot = fcio_pool.tile([128, 512], f16, tag="h4",
                                        name=f"h4_{m}", bufs=4)
                    relu_evac(ot[:], ps[:])
                    h4.append(ot)
                yt = fcio_pool.tile([128, 40], f32, tag="yout",
                                    name="yt", bufs=1)
                for b4 in range(4):
                    ps = fc3ps.tile([128, 10], f32, tag="fc3",
                                    name=f"fc3ps_{b4}")
                    for k in range(4):
                        nc.tensor.matmul(
                            ps[:],
                            h4[k][:, 128*b4:128*b4+128],
                            w3[:, 10*k:10*k+10],
                            start=(k == 0), stop=(k == 3))
                    nc.vector.tensor_copy(yt[:, 10*b4:10*b4+10], ps[:])
                nc.sync.dma_start(
                    out=y.ap().rearrange("(c p) o -> p c o", c=4),
                    in_=yt[:].rearrange("p (c o) -> p c o", c=4))
    nc.compile()
    return nc


def kernel(x, conv1w, conv2w, fc1, fc2, fc3):
    global LAST_EXEC_NS
    from concourse.bass_utils import run_bass_kernel_spmd

    x = np.ascontiguousarray(np.asarray(x, dtype=np.float32))
    conv1w = np.ascontiguousarray(np.asarray(conv1w, dtype=np.float32))
    conv2w = np.ascontiguousarray(np.asarray(conv2w, dtype=np.float32))
    fc1 = np.ascontiguousarray(np.asarray(fc1, dtype=np.float32))
    fc2 = np.ascontiguousarray(np.asarray(fc2, dtype=np.float32))
    fc3 = np.ascontiguousarray(np.asarray(fc3, dtype=np.float32))

    if not _NC_CACHE:
        _NC_CACHE.append(_build_nc())
    nc = _NC_CACHE[0]

    xpp = _prep_x(x)
    w2pm, w2sm = _prep_w2(conv2w)
    shared = {
        "w1t": _prep_w1(conv1w),
        "w2p": w2pm,
        "w2s": w2sm,
        "fc1m": _prep_fc1(fc1),
        "fc2t": _prep_fc2(fc2),
        "fc3t": _prep_fc3(fc3),
    }
    in_maps = [{**shared, "x_pp": xpp[c]} for c in range(N_CORES)]
    res = run_bass_kernel_spmd(nc, in_maps, list(range(N_CORES)))
    LAST_EXEC_NS = res.exec_time_ns
    return np.concatenate([r["y"] for r in res.results], axis=0)


# revision 5
# speedup vs baseline: 1.2144x; 1.0417x over previous
"""TRN2 Bass kernel for nn_CIFAR10_Type1_Template_Unroll (dense_cnn).

Network (per reference): two locally-connected conv layers + 3-layer FC
head, B=4096, fp32 in/out. Pure data parallel over 8 NeuronCores (512
batch each), activations on-chip in [feature, batch] layout, batch N=512
on the matmul free dim throughout. All DMA'd operands are fp16 (inputs
are O(1) normals; rounding is ~5e-4 relative, budget is 2e-2), halving
HBM traffic vs fp32.

DMA issue is serialized on the Sync engine at ~0.6us per dma_start, so
every input tensor is stored partition-major in DRAM and loaded with
~30 coarse transfers (0.25-1.6MB), ordered to match the consumption
order of the compute pipeline. fc1 weights are split: half prefetched
during phase 1, half loaded at phase-2 start into SBUF space freed by
the phase-1 pools, hidden under the first FC1 m-blocks' matmuls.

Layer mapping per core:
- L1 (k=2,s=2 locally-connected): host packs, per output row r and pair
  of adjacent positions, a K=32 strip (2 positions x 16 feats: 12 real +
  4 zero-pad) and a block-diagonal [32, 128] weight tile. 4 strips run
  concurrently on the PE's 32-row groups via tile_position=(32i, 0),
  writing two 2-bank PSUM doubles that are evacuated with single
  [128,1024] relu ops (evac cost scales with free dim only, so merging
  banks halves the per-bank cost; PSUM-source evacs run at 1x).
- L2 (k=4,s=2): positions are paired (h,2j)+(h,2j+1). The two positions
  share the middle input pair (2j+1), so a host-packed block weight
  [128,128] lets ONE full-width matmul per kh start the accumulation
  group for BOTH positions in one PSUM bank; the outer input pairs run
  as M=64 chains on PE column strips (0,0)/(0,64). One [128,512] relu
  evac per pair instead of two [64,512] halves. Column-7 positions pair
  across rows on column strips with separate banks (no shared rhs).
- FC head: K/M tiling, fc weights host-permuted to match the on-chip
  feature order of h2 ([pos-pair, parity, channel]).
A 16-matmul warmup on a memset scratch tile (no DMA dependency) ramps
the PE HAM clock gate from t~0 and bridges the first x-chunk DMA.
"""
import sys

if '/opt/trn_rl_repo' not in sys.path:
    sys.path.insert(0, '/opt/trn_rl_repo')

import numpy as np

N_CORES = 8
BS = 512
LAST_EXEC_NS = None

# ----------------------------------------------------------------- host prep

def _prep_x(x):
    """x [B,3,32,32] -> [N_CORES, 128, 16*2*512] fp16, partition-major.

    part = 32*i + 16*q + f; col = (r*2+g)*512 + b; pair p=4g+i covers
    w1 in {2p,2p+1}; q = w1 parity; f = c*4+kh*2+kw (12..15 zero-pad).
    """
    ncr = x.shape[0] // BS
    xr = x.reshape(ncr, BS, 3, 16, 2, 2, 4, 2, 2)   # s,b,c,r,kh,g,i,q,kw
    xt = xr.transpose(0, 3, 5, 6, 7, 2, 4, 8, 1)    # s,r,g,i,q,c,kh,kw,b
    xt = xt.reshape(ncr, 16, 2, 4, 2, 12, BS)
    xpp = np.zeros((ncr, 16, 2, 4, 2, 16, BS), np.float16)
    xpp[..., :12, :] = xt
    xpp = xpp.reshape(ncr, 16 * 2, 128, BS).transpose(0, 2, 1, 3)
    return np.ascontiguousarray(xpp.reshape(ncr, 128, 16 * 2 * BS))


def _prep_w1(conv1w):
    """conv1w [64,256,3,2,2] -> [128, 16*256] fp16 block-diag strips."""
    w1r = conv1w.reshape(64, 16, 16, 3, 2, 2)
    wt = w1r.transpose(1, 2, 3, 4, 5, 0).reshape(16, 16, 12, 64)
    wtp = np.zeros((16, 16, 16, 64), np.float32)
    wtp[:, :, :12, :] = wt
    wtp = wtp.reshape(16, 2, 4, 2, 16, 64)          # r,g,i,qp,f,o
    w1t = np.zeros((16, 2, 4, 2, 16, 2, 64), np.float32)
    w1t[:, :, :, 0, :, 0, :] = wtp[:, :, :, 0, :, :]
    w1t[:, :, :, 1, :, 1, :] = wtp[:, :, :, 1, :, :]
    w1t = w1t.reshape(16, 2, 128, 128).transpose(2, 0, 1, 3)  # p,r,g,c
    return np.ascontiguousarray(w1t.reshape(128, 16 * 256)).astype(np.float16)


def _prep_w2(conv2w):
    """conv2w [64,49,64,4,4] -> main [128,21*1024] + col-6 [128,7*512].

    Main pair T=(h, w'=2j / 2j+1): both positions read the middle input
    pair 2j+1, so cols 0:512 of pair block T hold, per kh, the
    [128,128] combined block (A's t=1 | B's t=0); cols 512:768 = A's
    left-pair (t=0) M=64 blocks; 768:1024 = B's right-pair (t=1).
    Partition = (q=kw%2)*64 + c, matching the L1 output tile layout.
    """
    wr = conv2w.reshape(64, 7, 7, 64, 4, 2, 2)      # o,h,w',c,kh,t,q
    wA = wr[:, :, 0:6:2]                            # o,h,j,c,kh,t,q
    wB = wr[:, :, 1:6:2]
    amid = wA[:, :, :, :, :, 1, :].transpose(1, 2, 5, 3, 4, 0)
    aleft = wA[:, :, :, :, :, 0, :].transpose(1, 2, 5, 3, 4, 0)
    bmid = wB[:, :, :, :, :, 0, :].transpose(1, 2, 5, 3, 4, 0)
    bright = wB[:, :, :, :, :, 1, :].transpose(1, 2, 5, 3, 4, 0)
    comb = np.stack([amid, bmid], axis=5)           # h,j,q,c,kh,half,o
    comb = comb.reshape(7, 3, 2, 64, 512)
    w2p = np.concatenate(
        [comb, aleft.reshape(7, 3, 2, 64, 256),
         bright.reshape(7, 3, 2, 64, 256)], axis=-1)   # h,j,q,c,1024
    w2p = w2p.transpose(2, 3, 0, 1, 4).reshape(128, 21 * 1024)
    w6 = wr[:, :, 6]                                # o,h,c,kh,t,q
    w2s = w6.transpose(5, 2, 1, 3, 4, 0).reshape(128, 7 * 512)
    return (np.ascontiguousarray(w2p).astype(np.float16),
            np.ascontiguousarray(w2s).astype(np.float16))


def _h2_posmap():
    pm = np.full((25, 2), -1, np.int64)
    for T in range(21):
        rr, j = divmod(T, 3)
        pm[T, 0] = rr * 7 + 2 * j
        pm[T, 1] = rr * 7 + 2 * j + 1
    for pi in range(4):
        r0, r1 = 2 * pi, 2 * pi + 1
        pm[21 + pi, 0] = r0 * 7 + 6
        if r1 < 7:
            pm[21 + pi, 1] = r1 * 7 + 6
    return pm


def _prep_fc1(fc1):
    pm = _h2_posmap()
    fc1p = fc1.reshape(1024, 64, 49)
    fc1hat = np.zeros((1024, 25, 2, 64), np.float32)
    for T in range(25):
        for u in range(2):
            p = pm[T, u]
            if p >= 0:
                fc1hat[:, T, u, :] = fc1p[:, :, p]
    a = fc1hat.reshape(8, 128, 25, 128).transpose(3, 0, 2, 1)  # kp,m,k,mc
    return np.ascontiguousarray(a.reshape(128, 8 * 3200)).astype(np.float16)


def _prep_fc2(fc2):
    a = fc2.reshape(4, 128, 8, 128).transpose(3, 0, 2, 1)      # kp,m,k,mc
    return np.ascontiguousarray(a.reshape(128, 4 * 1024)).astype(np.float16)


def _prep_fc3(fc3):
    a = fc3.T.reshape(4, 128, 10).transpose(1, 0, 2)
    return np.ascontiguousarray(a.reshape(128, 40)).astype(np.float16)


# --------------------------------------------------------------- bass kernel

_NC_CACHE = []


def _build_nc():
    import concourse.bass as bass
    import concourse.mybir as mybir
    from concourse import bacc
    from concourse.tile import TileContext

    f32 = mybir.dt.float32
    f16 = mybir.dt.float16
    RELU = mybir.ActivationFunctionType.Relu

    nc = bacc.Bacc("TRN2", target_bir_lowering=False, debug=False,
                   num_devices=N_CORES)
    x_pp = nc.dram_tensor("x_pp", [128, 16 * 2 * BS], f16,
                          kind="ExternalInput")
    w1t = nc.dram_tensor("w1t", [128, 16 * 256], f16, kind="ExternalInput")
    w2p = nc.dram_tensor("w2p", [128, 21 * 1024], f16, kind="ExternalInput")
    w2s = nc.dram_tensor("w2s", [128, 7 * 512], f16, kind="ExternalInput")
    fc1m = nc.dram_tensor("fc1m", [128, 8 * 3200], f16, kind="ExternalInput")
    fc2t = nc.dram_tensor("fc2t", [128, 4 * 1024], f16, kind="ExternalInput")
    fc3t = nc.dram_tensor("fc3t", [128, 40], f16, kind="ExternalInput")
    y = nc.dram_tensor("y", [BS, 10], f32, kind="ExternalOutput")

    ectr = [0]

    with TileContext(nc) as tc:
        def relu_evac(dst, src):
            if ectr[0] % 2 == 0:
                nc.scalar.activation(dst, src, RELU)
            else:
                nc.vector.tensor_scalar_max(dst, src, 0.0)
            ectr[0] += 1

        with (
            tc.tile_pool(name="h2pool", bufs=25) as h2pool,
            tc.tile_pool(name="fcw", bufs=1) as fcw_pool,
            tc.tile_pool(name="scratch", bufs=1) as sc_pool,
        ):
            h2 = [h2pool.tile([128, 512], f16, tag="h2", name=f"h2_{T}")
                  for T in range(25)]
            ws = sc_pool.tile([32, 512], f16, tag="ws", name="ws")
            fc1wA = fcw_pool.tile([128, 4 * 3200], f16, tag="fc1wA",
                                  name="fc1wA")
            # --------------- phase 1: L1 + L2 interleaved ---------------
            with (
                tc.tile_pool(name="xp", bufs=1) as xpp_pool,
                tc.tile_pool(name="w1p", bufs=1) as w1_pool,
                tc.tile_pool(name="w2pp", bufs=3) as w2p_pool,
                tc.tile_pool(name="w2sp", bufs=1) as w2s_pool,
                tc.tile_pool(name="o1p", bufs=28) as o1_pool,
                tc.tile_pool(name="l1ps", bufs=2, space="PSUM") as l1ps,
                tc.tile_pool(name="l2ps", bufs=4, space="PSUM") as l2ps,
            ):
                # PE warmup with no DMA dependency: memset scratch, then
                # dummy MMs so HAM un-throttles while the first x chunk
                # streams in.
                nc.gpsimd.memset(ws[:], 0.0)
                nc.vector.memset(h2[24][64:128, :], 0.0)
                wps = l2ps.tile([128, 512], f32, tag="l2", name="warm_ps")
                for wi in range(16):
                    nc.tensor.matmul(wps[:], ws[:, 0:128], ws[:],
                                     start=True, stop=True,
                                     tile_position=(0, 0))

                # resident x + w1; w2 main pairs stream per-pass
                xall = xpp_pool.tile([128, 16 * 1024], f16, tag="xa",
                                     name="xall")
                w1a = w1_pool.tile([128, 16 * 256], f16, tag="w1",
                                   name="w1a")
                w2sa = w2s_pool.tile([128, 7 * 512], f16, tag="w2s",
                                     name="w2sa")
                w2c = []

                def dma_w2c(h):
                    t = w2p_pool.tile([128, 3072], f16, tag="w2c",
                                      name=f"w2c_{h}")
                    nc.sync.dma_start(out=t[:],
                                      in_=w2p.ap()[:, 3072*h:3072*h+3072])
                    w2c.append(t)

                def dma_x(c0, c1):
                    nc.sync.dma_start(out=xall[:, c0:c1],
                                      in_=x_pp.ap()[:, c0:c1])

                # issue order = HBM service order (HWDGE FIFO): match the
                # compute pipeline's consumption order.
                for i in range(4):   # rows 0-1 split by PE row-strip
                    nc.sync.dma_start(out=xall[32*i:32*i+32, 0:2048],
                                      in_=x_pp.ap()[32*i:32*i+32, 0:2048])
                nc.sync.dma_start(out=w1a[:, 0:1024],
                                  in_=w1t.ap()[:, 0:1024])
                dma_x(2048, 3072)    # row 2
                dma_x(3072, 4096)    # row 3
                dma_w2c(0)
                dma_x(4096, 5120)    # row 4
                dma_x(5120, 6144)    # row 5
                dma_w2c(1)
                nc.sync.dma_start(out=w2sa[:], in_=w2s.ap())
                dma_x(6144, 7168)    # row 6
                dma_x(7168, 8192)    # row 7
                nc.sync.dma_start(out=w1a[:, 1024:4096],
                                  in_=w1t.ap()[:, 1024:4096])
                dma_w2c(2)
                dma_x(8192, 12288)   # rows 8-11
                dma_w2c(3)
                dma_x(12288, 16384)  # rows 12-15
                dma_w2c(4)
                nc.sync.dma_start(out=fc1wA[:, 0:6400],
                                  in_=fc1m.ap()[:, 0:6400])
                dma_w2c(5)
                dma_w2c(6)
                nc.sync.dma_start(out=fc1wA[:, 6400:12800],
                                  in_=fc1m.ap()[:, 6400:12800])

                out1d = [[None] * 4 for _ in range(16)]

                def out1(r, p):
                    u = p % 2
                    d = p // 2
                    base = 512 * u
                    return out1d[r][d][:, base:base + 512]

                def xsl(r, g, i):
                    c0 = 1024 * r + 512 * g
                    return xall[32*i:32*i+32, c0:c0 + 512]

                def w1sl(r, g, i):
                    c0 = r * 256 + g * 128
                    return w1a[32*i:32*i+32, c0:c0 + 128]

                def emit_l1_row(r):
                    for g in range(2):
                        pss = []
                        for d in range(2):
                            ps = l1ps.tile([128, 1024], f32, tag="l1",
                                           name=f"l1ps_{r}_{g}_{d}")
                            pss.append(ps)
                            for u in range(2):
                                i = 2 * d + u
                                nc.tensor.matmul(
                                    ps[:, 512*u:512*u+512],
                                    w1sl(r, g, i), xsl(r, g, i),
                                    start=True, stop=True,
                                    tile_position=(32 * i, 0))
                        for d in range(2):
                            ot = o1_pool.tile([128, 1024], f16, tag="o1",
                                              name=f"o1_{r}_{g}_{d}")
                            relu_evac(ot[:], pss[d][:])
                            out1d[r][2 * g + d] = ot

                def emit_main_pair(h, j):
                    T = 3 * h + j
                    wt = w2c[h][:, 1024*j:1024*j+1024]
                    ps = l2ps.tile([128, 512], f32, tag="l2",
                                   name=f"l2ps_{T}")
                    # combined full-width chain on the shared middle pair
                    for kh in range(4):
                        nc.tensor.matmul(
                            ps[:], wt[:, 128*kh:128*kh+128],
                            out1(2*h + kh, 2*j + 1),
                            start=(kh == 0), stop=False,
                            tile_position=(0, 0))
                    # outer pairs: A (left) on cols 0-63, B (right) on
                    # cols 64-127, interleaved for column-strip overlap
                    for kh in range(4):
                        nc.tensor.matmul(
                            ps[0:64, :], wt[:, 512+64*kh:512+64*kh+64],
                            out1(2*h + kh, 2*j),
                            start=False, stop=(kh == 3),
                            tile_position=(0, 0))
                        nc.tensor.matmul(
                            ps[64:128, :], wt[:, 768+64*kh:768+64*kh+64],
                            out1(2*h + kh, 2*j + 2),
                            start=False, stop=(kh == 3),
                            tile_position=(0, 64))
                    relu_evac(h2[T][:], ps[:])

                def emit_cross_pair(pi):
                    # positions (2pi, 6) and (2pi+1, 6) on column strips,
                    # separate PSUM banks (no shared rhs tile)
                    T = 21 + pi
                    hA, hB = 2 * pi, 2 * pi + 1
                    wA = w2sa[:, 512 * hA:512 * hA + 512]
                    wB = w2sa[:, 512 * hB:512 * hB + 512] if hB < 7 else None
                    psA = l2ps.tile([128, 512], f32, tag="l2",
                                    name=f"l2psA_{T}")
                    psB = None
                    if wB is not None:
                        psB = l2ps.tile([128, 512], f32, tag="l2",
                                        name=f"l2psB_{T}")
                    for kt in range(8):
                        kh, t = divmod(kt, 2)
                        nc.tensor.matmul(
                            psA[0:64, :], wA[:, 64*kt:64*kt+64],
                            out1(2*hA + kh, 6 + t),
                            start=(kt == 0), stop=(kt == 7),
                            tile_position=(0, 0))
                        if wB is not None:
                            nc.tensor.matmul(
                                psB[64:128, :], wB[:, 64*kt:64*kt+64],
                                out1(2*hB + kh, 6 + t),
                                start=(kt == 0), stop=(kt == 7),
                                tile_position=(0, 64))
                    relu_evac(h2[T][0:64, :], psA[0:64, :])
                    if wB is not None:
                        relu_evac(h2[T][64:128, :], psB[64:128, :])

                def emit_pass(h):
                    for j in range(3):
                        emit_main_pair(h, j)
                    if h in (1, 3, 5):
                        emit_cross_pair((h - 1) // 2)
                    if h == 6:
                        emit_cross_pair(3)

                for r in range(16):
                    emit_l1_row(r)
                    if r >= 4 and r % 2 == 0:
                        emit_pass((r - 4) // 2)
                emit_pass(6)

            # --------------- phase 2: FC head ---------------
            with (
                tc.tile_pool(name="fcio", bufs=12) as fcio_pool,
                tc.tile_pool(name="fcw2", bufs=1) as fcw2_pool,
                tc.tile_pool(name="fcps", bufs=2, space="PSUM") as fcps,
                tc.tile_pool(name="fc3ps", bufs=2, space="PSUM") as fc3ps,
            ):
                # second fc1 half + fc2/fc3 load into SBUF space freed by
                # the phase-1 pools, hidden under FC1 m0-m3 matmuls
                fc1wB = fcw2_pool.tile([128, 4 * 3200], f16, tag="fc1wB",
                                       name="fc1wB")
                nc.sync.dma_start(out=fc1wB[:, 0:6400],
                                  in_=fc1m.ap()[:, 12800:19200])
                nc.sync.dma_start(out=fc1wB[:, 6400:12800],
                                  in_=fc1m.ap()[:, 19200:25600])
                fc2w = fcw2_pool.tile([128, 4 * 1024], f16, tag="fc2w",
                                      name="fc2w")
                nc.sync.dma_start(out=fc2w[:], in_=fc2t.ap())
                w3 = fcw2_pool.tile([128, 40], f16, tag="fc3w", name="fc3w")
                nc.sync.dma_start(out=w3[:], in_=fc3t.ap())
                h3 = []
                for m in range(8):
                    wsrc = (fc1wA if m < 4 else fc1wB)
                    mo = 3200 * (m % 4)
                    ps = fcps.tile([128, 512], f32, tag="fc",
                                   name=f"fc1ps_{m}")
                    for k in range(25):
                        nc.tensor.matmul(
                            ps[:], wsrc[:, mo+128*k:mo+128*k+128],
                            h2[k][:],
                            start=(k == 0), stop=(k == 24))
                    ot = fcio_pool.tile([128, 512], f16, tag="h3",
                                        name=f"h3_{m}", bufs=8)
                    relu_evac(ot[:], ps[:])
                    h3.append(ot)
                h4 = []
                for m in range(4):
                    ps = fcps.tile([128, 512], f32, tag="fc",
                                   name=f"fc2ps_{m}")
                    for k in range(8):
                        nc.tensor.matmul(
                            ps[:],
                            fc2w[:, 1024*m+128*k:1024*m+128*k+128],
                            h3[k][:],
                            start=(k == 0), stop=(k == 7))
                    ot = fcio_pool.tile([128, 512], f16, tag="h4",
                                        name=f"h4_{m}", bufs=4)
                    relu_evac(ot[:], ps[:])
                    h4.append(ot)
                yt = fcio_pool.tile([128, 40], f32, tag="yout",
                                    name="yt", bufs=1)
                for b4 in range(4):
                    ps = fc3ps.tile([128, 10], f32, tag="fc3",
                                    name=f"fc3ps_{b4}")
                    for k in range(4):
                        nc.tensor.matmul(
                            ps[:],
                            h4[k][:, 128*b4:128*b4+128],
                            w3[:, 10*k:10*k+10],
                            start=(k == 0), stop=(k == 3))
                    nc.vector.tensor_copy(yt[:, 10*b4:10*b4+10], ps[:])
                nc.sync.dma_start(
                    out=y.ap().rearrange("(c p) o -> p c o", c=4),
                    in_=yt[:].rearrange("p (c o) -> p c o", c=4))
    nc.compile()
    return nc


def kernel(x, conv1w, conv2w, fc1, fc2, fc3):
    global LAST_EXEC_NS
    from concourse.bass_utils import run_bass_kernel_spmd

    x = np.ascontiguousarray(np.asarray(x, dtype=np.float32))
    conv1w = np.ascontiguousarray(np.asarray(conv1w, dtype=np.float32))
    conv2w = np.ascontiguousarray(np.asarray(conv2w, dtype=np.float32))
    fc1 = np.ascontiguousarray(np.asarray(fc1, dtype=np.float32))
    fc2 = np.ascontiguousarray(np.asarray(fc2, dtype=np.float32))
    fc3 = np.ascontiguousarray(np.asarray(fc3, dtype=np.float32))

    if not _NC_CACHE:
        _NC_CACHE.append(_build_nc())
    nc = _NC_CACHE[0]

    xpp = _prep_x(x)
    w2pm, w2sm = _prep_w2(conv2w)
    shared = {
        "w1t": _prep_w1(conv1w),
        "w2p": w2pm,
        "w2s": w2sm,
        "fc1m": _prep_fc1(fc1),
        "fc2t": _prep_fc2(fc2),
        "fc3t": _prep_fc3(fc3),
    }
    in_maps = [{**shared, "x_pp": xpp[c]} for c in range(N_CORES)]
    res = run_bass_kernel_spmd(nc, in_maps, list(range(N_CORES)))
    LAST_EXEC_NS = res.exec_time_ns
    return np.concatenate([r["y"] for r in res.results], axis=0)


# revision 10
# speedup vs baseline: 1.2808x; 1.0547x over previous
"""TRN2 Bass kernel for nn_CIFAR10_Type1_Template_Unroll (dense_cnn).

Network (per reference): two locally-connected conv layers + 3-layer FC
head, B=4096, fp32 in/out. Pure data parallel over 8 NeuronCores (512
batch each), activations on-chip in [feature, batch] layout, batch N=512
on the matmul free dim throughout. All DMA'd operands are fp16 (inputs
are O(1) normals; rounding is ~5e-4 relative, budget is 2e-2), halving
HBM traffic vs fp32.

DMA issue is serialized on the Sync engine at ~0.6us per dma_start, so
every input tensor is stored partition-major in DRAM and loaded with
~30 coarse transfers (0.25-1.6MB), ordered to match the consumption
order of the compute pipeline. fc1 weights are split: half prefetched
during phase 1, half loaded at phase-2 start into SBUF space freed by
the phase-1 pools, hidden under the first FC1 m-blocks' matmuls.

Layer mapping per core:
- L1 (k=2,s=2 locally-connected): host packs, per output row r and pair
  of adjacent positions, a K=32 strip (2 positions x 16 feats: 12 real +
  4 zero-pad) and a block-diagonal [32, 128] weight tile. 4 strips run
  concurrently on the PE's 32-row groups via tile_position=(32i, 0),
  writing two 2-bank PSUM doubles that are evacuated with single
  [128,1024] relu ops (evac cost scales with free dim only, so merging
  banks halves the per-bank cost; PSUM-source evacs run at 1x).
- L2 (k=4,s=2): positions are paired (h,2j)+(h,2j+1). The two positions
  share the middle input pair (2j+1), so a host-packed block weight
  [128,128] lets ONE full-width matmul per kh start the accumulation
  group for BOTH positions in one PSUM bank; the outer input pairs run
  as M=64 chains on PE column strips (0,0)/(0,64). One [128,512] relu
  evac per pair instead of two [64,512] halves. Column-7 positions pair
  across rows on column strips with separate banks (no shared rhs).
- FC head: K/M tiling, fc weights host-permuted to match the on-chip
  feature order of h2 ([pos-pair, parity, channel]).
A 16-matmul warmup on a memset scratch tile (no DMA dependency) ramps
the PE HAM clock gate from t~0 and bridges the first x-chunk DMA.
"""
import sys

if '/opt/trn_rl_repo' not in sys.path:
    sys.path.insert(0, '/opt/trn_rl_repo')

import numpy as np

N_CORES = 8
BS = 512
LAST_EXEC_NS = None

# ----------------------------------------------------------------- host prep

def _prep_x(x):
    """x [B,3,32,32] -> [N_CORES, 128, 16*2*512] fp16, partition-major.

    part = 32*i + 16*q + f; col = (r*2+g)*512 + b; pair p=4g+i covers
    w1 in {2p,2p+1}; q = w1 parity; f = c*4+kh*2+kw (12..15 zero-pad).
    """
    ncr = x.shape[0] // BS
    xr = x.reshape(ncr, BS, 3, 16, 2, 2, 4, 2, 2)   # s,b,c,r,kh,g,i,q,kw
    xt = xr.transpose(0, 3, 5, 6, 7, 2, 4, 8, 1)    # s,r,g,i,q,c,kh,kw,b
    xt = xt.reshape(ncr, 16, 2, 4, 2, 12, BS)
    xpp = np.zeros((ncr, 16, 2, 4, 2, 16, BS), np.float16)
    xpp[..., :12, :] = xt
    xpp = xpp.reshape(ncr, 16 * 2, 128, BS).transpose(0, 2, 1, 3)
    return np.ascontiguousarray(xpp.reshape(ncr, 128, 16 * 2 * BS))


def _prep_w1(conv1w):
    """conv1w [64,256,3,2,2] -> [128, 16*256] fp16 block-diag strips."""
    w1r = conv1w.reshape(64, 16, 16, 3, 2, 2)
    wt = w1r.transpose(1, 2, 3, 4, 5, 0).reshape(16, 16, 12, 64)
    wtp = np.zeros((16, 16, 16, 64), np.float32)
    wtp[:, :, :12, :] = wt
    wtp = wtp.reshape(16, 2, 4, 2, 16, 64)          # r,g,i,qp,f,o
    w1t = np.zeros((16, 2, 4, 2, 16, 2, 64), np.float32)
    w1t[:, :, :, 0, :, 0, :] = wtp[:, :, :, 0, :, :]
    w1t[:, :, :, 1, :, 1, :] = wtp[:, :, :, 1, :, :]
    w1t = w1t.reshape(16, 2, 128, 128).transpose(2, 0, 1, 3)  # p,r,g,c
    return np.ascontiguousarray(w1t.reshape(128, 16 * 256)).astype(np.float16)


def _prep_w2(conv2w):
    """conv2w [64,49,64,4,4] -> main [128,21*1024] + col-6 [128,7*512].

    Main pair T=(h, w'=2j / 2j+1): both positions read the middle input
    pair 2j+1, so cols 0:512 of pair block T hold, per kh, the
    [128,128] combined block (A's t=1 | B's t=0); cols 512:768 = A's
    left-pair (t=0) M=64 blocks; 768:1024 = B's right-pair (t=1).
    Partition = (q=kw%2)*64 + c, matching the L1 output tile layout.
    """
    wr = conv2w.reshape(64, 7, 7, 64, 4, 2, 2)      # o,h,w',c,kh,t,q
    wA = wr[:, :, 0:6:2]                            # o,h,j,c,kh,t,q
    wB = wr[:, :, 1:6:2]
    amid = wA[:, :, :, :, :, 1, :].transpose(1, 2, 5, 3, 4, 0)
    aleft = wA[:, :, :, :, :, 0, :].transpose(1, 2, 5, 3, 4, 0)
    bmid = wB[:, :, :, :, :, 0, :].transpose(1, 2, 5, 3, 4, 0)
    bright = wB[:, :, :, :, :, 1, :].transpose(1, 2, 5, 3, 4, 0)
    comb = np.stack([amid, bmid], axis=5)           # h,j,q,c,kh,half,o
    comb = comb.reshape(7, 3, 2, 64, 512)
    w2p = np.concatenate(
        [comb, aleft.reshape(7, 3, 2, 64, 256),
         bright.reshape(7, 3, 2, 64, 256)], axis=-1)   # h,j,q,c,1024
    w2p = w2p.transpose(2, 3, 0, 1, 4).reshape(128, 21 * 1024)
    w6 = wr[:, :, 6]                                # o,h,c,kh,t,q
    w2s = w6.transpose(5, 2, 1, 3, 4, 0).reshape(128, 7 * 512)
    return (np.ascontiguousarray(w2p).astype(np.float16),
            np.ascontiguousarray(w2s).astype(np.float16))


def _h2_posmap():
    pm = np.full((25, 2), -1, np.int64)
    for T in range(21):
        rr, j = divmod(T, 3)
        pm[T, 0] = rr * 7 + 2 * j
        pm[T, 1] = rr * 7 + 2 * j + 1
    for pi in range(4):
        r0, r1 = 2 * pi, 2 * pi + 1
        pm[21 + pi, 0] = r0 * 7 + 6
        if r1 < 7:
            pm[21 + pi, 1] = r1 * 7 + 6
    return pm


def _prep_fc1(fc1):
    pm = _h2_posmap()
    fc1p = fc1.reshape(1024, 64, 49)
    fc1hat = np.zeros((1024, 25, 2, 64), np.float32)
    for T in range(25):
        for u in range(2):
            p = pm[T, u]
            if p >= 0:
                fc1hat[:, T, u, :] = fc1p[:, :, p]
    a = fc1hat.reshape(8, 128, 25, 128).transpose(3, 0, 2, 1)  # kp,m,k,mc
    return np.ascontiguousarray(a.reshape(128, 8 * 3200)).astype(np.float16)


def _prep_fc2(fc2):
    a = fc2.reshape(4, 128, 8, 128).transpose(3, 0, 2, 1)      # kp,m,k,mc
    return np.ascontiguousarray(a.reshape(128, 4 * 1024)).astype(np.float16)


def _prep_fc3(fc3):
    a = fc3.T.reshape(4, 128, 10).transpose(1, 0, 2)
    return np.ascontiguousarray(a.reshape(128, 40)).astype(np.float16)


# --------------------------------------------------------------- bass kernel

_NC_CACHE = []


def _build_nc():
    import concourse.bass as bass
    import concourse.mybir as mybir
    from concourse import bacc
    from concourse.tile import TileContext

    f32 = mybir.dt.float32
    f16 = mybir.dt.float16
    RELU = mybir.ActivationFunctionType.Relu

    nc = bacc.Bacc("TRN2", target_bir_lowering=False, debug=False,
                   num_devices=N_CORES)
    x_pp = nc.dram_tensor("x_pp", [128, 16 * 2 * BS], f16,
                          kind="ExternalInput")
    w1t = nc.dram_tensor("w1t", [128, 16 * 256], f16, kind="ExternalInput")
    w2p = nc.dram_tensor("w2p", [128, 21 * 1024], f16, kind="ExternalInput")
    w2s = nc.dram_tensor("w2s", [128, 7 * 512], f16, kind="ExternalInput")
    fc1m = nc.dram_tensor("fc1m", [128, 8 * 3200], f16, kind="ExternalInput")
    fc2t = nc.dram_tensor("fc2t", [128, 4 * 1024], f16, kind="ExternalInput")
    fc3t = nc.dram_tensor("fc3t", [128, 40], f16, kind="ExternalInput")
    y = nc.dram_tensor("y", [BS, 10], f32, kind="ExternalOutput")

    ectr = [0]

    with TileContext(nc) as tc:
        def relu_evac(dst, src):
            if ectr[0] % 2 == 0:
                nc.scalar.activation(dst, src, RELU)
            else:
                nc.vector.tensor_scalar_max(dst, src, 0.0)
            ectr[0] += 1

        with (
            tc.tile_pool(name="h2pool", bufs=25) as h2pool,
            tc.tile_pool(name="fcw", bufs=1) as fcw_pool,
            tc.tile_pool(name="scratch", bufs=1) as sc_pool,
        ):
            h2 = [h2pool.tile([128, 512], f16, tag="h2", name=f"h2_{T}")
                  for T in range(25)]
            ws = sc_pool.tile([128, 512], f16, tag="ws", name="ws")
            fc1wA = fcw_pool.tile([128, 4 * 3200], f16, tag="fc1wA",
                                  name="fc1wA")
            # --------------- phase 1: L1 + L2 interleaved ---------------
            with (
                tc.tile_pool(name="xp", bufs=1) as xpp_pool,
                tc.tile_pool(name="w1p", bufs=1) as w1_pool,
                tc.tile_pool(name="w2pp", bufs=3) as w2p_pool,
                tc.tile_pool(name="w2sp", bufs=1) as w2s_pool,
                tc.tile_pool(name="o1p", bufs=32) as o1_pool,
                tc.tile_pool(name="l1ps", bufs=2, space="PSUM") as l1ps,
                tc.tile_pool(name="l2ps", bufs=4, space="PSUM") as l2ps,
            ):
                # PE warmup with no DMA dependency: memset scratch, then
                # dummy MMs so HAM un-throttles while the first x chunk
                # streams in. K=128 full-row matmuls: HAM's activity
                # monitor does not register K=32 row-strip matmuls as
                # busy (observed: the clock stays at 4/8 until ~3.4us
                # after the first sustained full-row matmuls).
                nc.gpsimd.memset(ws[:], 0.0)
                nc.vector.memset(h2[24][64:128, :], 0.0)
                wps = l2ps.tile([128, 512], f32, tag="l2", name="warm_ps")
                for wi in range(12):
                    nc.tensor.matmul(wps[:], ws[:, 0:128], ws[:],
                                     start=True, stop=True)

                # resident x + w1; w2 main pairs stream per-pass
                xall = xpp_pool.tile([128, 16 * 1024], f16, tag="xa",
                                     name="xall")
                w1a = w1_pool.tile([128, 16 * 256], f16, tag="w1",
                                   name="w1a")
                w2sa = w2s_pool.tile([128, 7 * 512], f16, tag="w2s",
                                     name="w2sa")
                w2c = []

                def dma_w2c(h):
                    t = w2p_pool.tile([128, 3072], f16, tag="w2c",
                                      name=f"w2c_{h}")
                    nc.sync.dma_start(out=t[:],
                                      in_=w2p.ap()[:, 3072*h:3072*h+3072])
                    w2c.append(t)

                def dma_x(c0, c1):
                    nc.sync.dma_start(out=xall[:, c0:c1],
                                      in_=x_pp.ap()[:, c0:c1])

                # issue order = HBM service order (HWDGE FIFO): match the
                # compute pipeline's consumption order.
                for i in range(4):   # rows 0-1 split by PE row-strip
                    nc.sync.dma_start(out=xall[32*i:32*i+32, 0:2048],
                                      in_=x_pp.ap()[32*i:32*i+32, 0:2048])
                nc.sync.dma_start(out=w1a[:, 0:1024],
                                  in_=w1t.ap()[:, 0:1024])
                dma_x(2048, 3072)    # row 2
                dma_x(3072, 4096)    # row 3
                dma_w2c(0)
                dma_x(4096, 5120)    # row 4
                dma_x(5120, 6144)    # row 5
                dma_w2c(1)
                nc.sync.dma_start(out=w2sa[:], in_=w2s.ap())
                dma_x(6144, 7168)    # row 6
                dma_x(7168, 8192)    # row 7
                nc.sync.dma_start(out=w1a[:, 1024:4096],
                                  in_=w1t.ap()[:, 1024:4096])
                dma_w2c(2)
                dma_x(8192, 12288)   # rows 8-11
                dma_w2c(3)
                dma_x(12288, 16384)  # rows 12-15
                dma_w2c(4)
                nc.sync.dma_start(out=fc1wA[:, 0:6400],
                                  in_=fc1m.ap()[:, 0:6400])
                dma_w2c(5)
                dma_w2c(6)
                nc.sync.dma_start(out=fc1wA[:, 6400:12800],
                                  in_=fc1m.ap()[:, 6400:12800])

                out1d = [[None] * 4 for _ in range(16)]

                def out1(r, p):
                    u = p % 2
                    d = p // 2
                    base = 512 * u
                    return out1d[r][d][:, base:base + 512]

                def xsl(r, g, i):
                    c0 = 1024 * r + 512 * g
                    return xall[32*i:32*i+32, c0:c0 + 512]

                def w1sl(r, g, i):
                    c0 = r * 256 + g * 128
                    return w1a[32*i:32*i+32, c0:c0 + 128]

                def emit_l1_group(r, g):
                    pss = []
                    for d in range(2):
                        ps = l1ps.tile([128, 1024], f32, tag="l1",
                                       name=f"l1ps_{r}_{g}_{d}")
                        pss.append(ps)
                        for u in range(2):
                            i = 2 * d + u
                            nc.tensor.matmul(
                                ps[:, 512*u:512*u+512],
                                w1sl(r, g, i), xsl(r, g, i),
                                start=True, stop=True,
                                tile_position=(32 * i, 0))
                    for d in range(2):
                        ot = o1_pool.tile([128, 1024], f16, tag="o1",
                                          name=f"o1_{r}_{g}_{d}")
                        relu_evac(ot[:], pss[d][:])
                        out1d[r][2 * g + d] = ot

                def emit_main_pair(h, j):
                    T = 3 * h + j
                    wt = w2c[h][:, 1024*j:1024*j+1024]
                    ps = l2ps.tile([128, 512], f32, tag="l2",
                                   name=f"l2ps_{T}")
                    # combined full-width chain on the shared middle pair
                    for kh in range(4):
                        nc.tensor.matmul(
                            ps[:], wt[:, 128*kh:128*kh+128],
                            out1(2*h + kh, 2*j + 1),
                            start=(kh == 0), stop=False,
                            tile_position=(0, 0))
                    # outer pairs: A (left) on cols 0-63, B (right) on
                    # cols 64-127, interleaved for column-strip overlap
                    for kh in range(4):
                        nc.tensor.matmul(
                            ps[0:64, :], wt[:, 512+64*kh:512+64*kh+64],
                            out1(2*h + kh, 2*j),
                            start=False, stop=(kh == 3),
                            tile_position=(0, 0))
                        nc.tensor.matmul(
                            ps[64:128, :], wt[:, 768+64*kh:768+64*kh+64],
                            out1(2*h + kh, 2*j + 2),
                            start=False, stop=(kh == 3),
                            tile_position=(0, 64))
                    relu_evac(h2[T][:], ps[:])

                def emit_cross_pair(pi):
                    # positions (2pi, 6) and (2pi+1, 6) on column strips,
                    # separate PSUM banks (no shared rhs tile)
                    T = 21 + pi
                    hA, hB = 2 * pi, 2 * pi + 1
                    wA = w2sa[:, 512 * hA:512 * hA + 512]
                    wB = w2sa[:, 512 * hB:512 * hB + 512] if hB < 7 else None
                    psA = l2ps.tile([128, 512], f32, tag="l2",
                                    name=f"l2psA_{T}")
                    psB = None
                    if wB is not None:
                        psB = l2ps.tile([128, 512], f32, tag="l2",
                                        name=f"l2psB_{T}")
                    for kt in range(8):
                        kh, t = divmod(kt, 2)
                        nc.tensor.matmul(
                            psA[0:64, :], wA[:, 64*kt:64*kt+64],
                            out1(2*hA + kh, 6 + t),
                            start=(kt == 0), stop=(kt == 7),
                            tile_position=(0, 0))
                        if wB is not None:
                            nc.tensor.matmul(
                                psB[64:128, :], wB[:, 64*kt:64*kt+64],
                                out1(2*hB + kh, 6 + t),
                                start=(kt == 0), stop=(kt == 7),
                                tile_position=(0, 64))
                    relu_evac(h2[T][0:64, :], psA[0:64, :])
                    if wB is not None:
                        relu_evac(h2[T][64:128, :], psB[64:128, :])

                # lead-in: rows 0-3 (pass 0 depends on them)
                for r in range(4):
                    for g in range(2):
                        emit_l1_group(r, g)
                # steady state: interleave pass-h L2 chains with the L1
                # groups of rows 2h+4/2h+5 so every l1ps double's evac
                # hides under an L2 chain (l1ps has only 2 slots)
                for h in range(6):
                    chains = [lambda j=j: emit_main_pair(h, j)
                              for j in range(3)]
                    if h in (1, 3, 5):
                        chains.append(lambda: emit_cross_pair((h - 1) // 2))
                    groups = [(2*h + 4, 0), (2*h + 4, 1),
                              (2*h + 5, 0), (2*h + 5, 1)]
                    for k in range(4):
                        if k < len(chains):
                            chains[k]()
                        emit_l1_group(*groups[k])
                for j in range(3):
                    emit_main_pair(6, j)
                emit_cross_pair(3)

            # --------------- phase 2: FC head ---------------
            with (
                tc.tile_pool(name="fcio", bufs=12) as fcio_pool,
                tc.tile_pool(name="fcw2", bufs=1) as fcw2_pool,
                tc.tile_pool(name="fcps", bufs=2, space="PSUM") as fcps,
                tc.tile_pool(name="fc3ps", bufs=2, space="PSUM") as fc3ps,
            ):
                # second fc1 half + fc2/fc3 load into SBUF space freed by
                # the phase-1 pools, hidden under FC1 m0-m3 matmuls
                fc1wB = fcw2_pool.tile([128, 4 * 3200], f16, tag="fc1wB",
                                       name="fc1wB")
                nc.sync.dma_start(out=fc1wB[:, 0:6400],
                                  in_=fc1m.ap()[:, 12800:19200])
                nc.sync.dma_start(out=fc1wB[:, 6400:12800],
                                  in_=fc1m.ap()[:, 19200:25600])
                fc2w = fcw2_pool.tile([128, 4 * 1024], f16, tag="fc2w",
                                      name="fc2w")
                nc.sync.dma_start(out=fc2w[:], in_=fc2t.ap())
                w3 = fcw2_pool.tile([128, 40], f16, tag="fc3w", name="fc3w")
                nc.sync.dma_start(out=w3[:], in_=fc3t.ap())
                h3 = []
                for m in range(8):
                    wsrc = (fc1wA if m < 4 else fc1wB)
                    mo = 3200 * (m % 4)
                    ps = fcps.tile([128, 512], f32, tag="fc",
                                   name=f"fc1ps_{m}")
                    for k in range(25):
                        nc.tensor.matmul(
                            ps[:], wsrc[:, mo+128*k:mo+128*k+128],
                            h2[k][:],
                            start=(k == 0), stop=(k == 24))
                    ot = fcio_pool.tile([128, 512], f16, tag="h3",
                                        name=f"h3_{m}", bufs=8)
                    relu_evac(ot[:], ps[:])
                    h3.append(ot)
                h4 = []
                for m in range(4):
                    ps = fcps.tile([128, 512], f32, tag="fc",
                                   name=f"fc2ps_{m}")
                    for k in range(8):
                        nc.tensor.matmul(
                            ps[:],
                            fc2w[:, 1024*m+128*k:1024*m+128*k+128],
                            h3[k][:],
                            start=(k == 0), stop=(k == 7))
                    ot = fcio_pool.tile([128, 512], f16, tag="h4",
                                        name=f"h4_{m}", bufs=4)
                    relu_evac(ot[:], ps[:])
                    h4.append(ot)
                yt = fcio_pool.tile([128, 40], f32, tag="yout",
                                    name="yt", bufs=1)
                for b4 in range(4):
                    ps = fc3ps.tile([128, 10], f32, tag="fc3",
                                    name=f"fc3ps_{b4}")
                    for k in range(4):
                        nc.tensor.matmul(
                            ps[:],
                            h4[k][:, 128*b4:128*b4+128],
                            w3[:, 10*k:10*k+10],
                            start=(k == 0), stop=(k == 3))
                    nc.vector.tensor_copy(yt[:, 10*b4:10*b4+10], ps[:])
                nc.sync.dma_start(
                    out=y.ap().rearrange("(c p) o -> p c o", c=4),
                    in_=yt[:].rearrange("p (c o) -> p c o", c=4))
    nc.compile()
    return nc


def kernel(x, conv1w, conv2w, fc1, fc2, fc3):
    global LAST_EXEC_NS
    from concourse.bass_utils import run_bass_kernel_spmd

    x = np.ascontiguousarray(np.asarray(x, dtype=np.float32))
    conv1w = np.ascontiguousarray(np.asarray(conv1w, dtype=np.float32))
    conv2w = np.ascontiguousarray(np.asarray(conv2w, dtype=np.float32))
    fc1 = np.ascontiguousarray(np.asarray(fc1, dtype=np.float32))
    fc2 = np.ascontiguousarray(np.asarray(fc2, dtype=np.float32))
    fc3 = np.ascontiguousarray(np.asarray(fc3, dtype=np.float32))

    if not _NC_CACHE:
        _NC_CACHE.append(_build_nc())
    nc = _NC_CACHE[0]

    xpp = _prep_x(x)
    w2pm, w2sm = _prep_w2(conv2w)
    shared = {
        "w1t": _prep_w1(conv1w),
        "w2p": w2pm,
        "w2s": w2sm,
        "fc1m": _prep_fc1(fc1),
        "fc2t": _prep_fc2(fc2),
        "fc3t": _prep_fc3(fc3),
    }
    in_maps = [{**shared, "x_pp": xpp[c]} for c in range(N_CORES)]
    res = run_bass_kernel_spmd(nc, in_maps, list(range(N_CORES)))
    LAST_EXEC_NS = res.exec_time_ns
    return np.concatenate([r["y"] for r in res.results], axis=0)


# revision 13
# speedup vs baseline: 1.2834x; 1.0020x over previous
"""TRN2 Bass kernel for nn_CIFAR10_Type1_Template_Unroll (dense_cnn).

Network (per reference): two locally-connected conv layers + 3-layer FC
head, B=4096, fp32 in/out. Pure data parallel over 8 NeuronCores (512
batch each), activations on-chip in [feature, batch] layout, batch N=512
on the matmul free dim throughout. All DMA'd operands are fp16 (inputs
are O(1) normals; rounding is ~5e-4 relative, budget is 2e-2), halving
HBM traffic vs fp32.

DMA issue is serialized on the Sync engine at ~0.6us per dma_start, so
every input tensor is stored partition-major in DRAM and loaded with
~30 coarse transfers (0.25-1.6MB), ordered to match the consumption
order of the compute pipeline. fc1 weights are split: half prefetched
during phase 1, half loaded at phase-2 start into SBUF space freed by
the phase-1 pools, hidden under the first FC1 m-blocks' matmuls.

Layer mapping per core:
- L1 (k=2,s=2 locally-connected): host packs, per output row r and pair
  of adjacent positions, a K=32 strip (2 positions x 16 feats: 12 real +
  4 zero-pad) and a block-diagonal [32, 128] weight tile. 4 strips run
  concurrently on the PE's 32-row groups via tile_position=(32i, 0),
  writing two 2-bank PSUM doubles that are evacuated with single
  [128,1024] relu ops (evac cost scales with free dim only, so merging
  banks halves the per-bank cost; PSUM-source evacs run at 1x).
- L2 (k=4,s=2): positions are paired (h,2j)+(h,2j+1). The two positions
  share the middle input pair (2j+1), so a host-packed block weight
  [128,128] lets ONE full-width matmul per kh start the accumulation
  group for BOTH positions in one PSUM bank; the outer input pairs run
  as M=64 chains on PE column strips (0,0)/(0,64). One [128,512] relu
  evac per pair instead of two [64,512] halves. Column-7 positions pair
  across rows on column strips with separate banks (no shared rhs).
- FC head: K/M tiling, fc weights host-permuted to match the on-chip
  feature order of h2 ([pos-pair, parity, channel]).
A 16-matmul warmup on a memset scratch tile (no DMA dependency) ramps
the PE HAM clock gate from t~0 and bridges the first x-chunk DMA.
"""
import sys

if '/opt/trn_rl_repo' not in sys.path:
    sys.path.insert(0, '/opt/trn_rl_repo')

import numpy as np

N_CORES = 8
BS = 512
LAST_EXEC_NS = None

# ----------------------------------------------------------------- host prep

def _prep_x(x):
    """x [B,3,32,32] -> [N_CORES, 128, 16*2*512] fp16, partition-major.

    part = 32*i + 16*q + f; col = (r*2+g)*512 + b; pair p=4g+i covers
    w1 in {2p,2p+1}; q = w1 parity; f = c*4+kh*2+kw (12..15 zero-pad).
    """
    ncr = x.shape[0] // BS
    xr = x.reshape(ncr, BS, 3, 16, 2, 2, 4, 2, 2)   # s,b,c,r,kh,g,i,q,kw
    xt = xr.transpose(0, 3, 5, 6, 7, 2, 4, 8, 1)    # s,r,g,i,q,c,kh,kw,b
    xt = xt.reshape(ncr, 16, 2, 4, 2, 12, BS)
    xpp = np.zeros((ncr, 16, 2, 4, 2, 16, BS), np.float16)
    xpp[..., :12, :] = xt
    xpp = xpp.reshape(ncr, 16 * 2, 128, BS).transpose(0, 2, 1, 3)
    return np.ascontiguousarray(xpp.reshape(ncr, 128, 16 * 2 * BS))


def _prep_w1(conv1w):
    """conv1w [64,256,3,2,2] -> [128, 16*256] fp16 block-diag strips."""
    w1r = conv1w.reshape(64, 16, 16, 3, 2, 2)
    wt = w1r.transpose(1, 2, 3, 4, 5, 0).reshape(16, 16, 12, 64)
    wtp = np.zeros((16, 16, 16, 64), np.float32)
    wtp[:, :, :12, :] = wt
    wtp = wtp.reshape(16, 2, 4, 2, 16, 64)          # r,g,i,qp,f,o
    w1t = np.zeros((16, 2, 4, 2, 16, 2, 64), np.float32)
    w1t[:, :, :, 0, :, 0, :] = wtp[:, :, :, 0, :, :]
    w1t[:, :, :, 1, :, 1, :] = wtp[:, :, :, 1, :, :]
    w1t = w1t.reshape(16, 2, 128, 128).transpose(2, 0, 1, 3)  # p,r,g,c
    return np.ascontiguousarray(w1t.reshape(128, 16 * 256)).astype(np.float16)


def _prep_w2(conv2w):
    """conv2w [64,49,64,4,4] -> main [128,21*1024] + col-6 [128,7*512].

    Main pair T=(h, w'=2j / 2j+1): both positions read the middle input
    pair 2j+1, so cols 0:512 of pair block T hold, per kh, the
    [128,128] combined block (A's t=1 | B's t=0); cols 512:768 = A's
    left-pair (t=0) M=64 blocks; 768:1024 = B's right-pair (t=1).
    Partition = (q=kw%2)*64 + c, matching the L1 output tile layout.
    """
    wr = conv2w.reshape(64, 7, 7, 64, 4, 2, 2)      # o,h,w',c,kh,t,q
    wA = wr[:, :, 0:6:2]                            # o,h,j,c,kh,t,q
    wB = wr[:, :, 1:6:2]
    amid = wA[:, :, :, :, :, 1, :].transpose(1, 2, 5, 3, 4, 0)
    aleft = wA[:, :, :, :, :, 0, :].transpose(1, 2, 5, 3, 4, 0)
    bmid = wB[:, :, :, :, :, 0, :].transpose(1, 2, 5, 3, 4, 0)
    bright = wB[:, :, :, :, :, 1, :].transpose(1, 2, 5, 3, 4, 0)
    comb = np.stack([amid, bmid], axis=5)           # h,j,q,c,kh,half,o
    comb = comb.reshape(7, 3, 2, 64, 512)
    w2p = np.concatenate(
        [comb, aleft.reshape(7, 3, 2, 64, 256),
         bright.reshape(7, 3, 2, 64, 256)], axis=-1)   # h,j,q,c,1024
    w2p = w2p.transpose(2, 3, 0, 1, 4).reshape(128, 21 * 1024)
    w6 = wr[:, :, 6]                                # o,h,c,kh,t,q
    w2s = w6.transpose(5, 2, 1, 3, 4, 0).reshape(128, 7 * 512)
    return (np.ascontiguousarray(w2p).astype(np.float16),
            np.ascontiguousarray(w2s).astype(np.float16))


def _h2_posmap():
    pm = np.full((25, 2), -1, np.int64)
    for T in range(21):
        rr, j = divmod(T, 3)
        pm[T, 0] = rr * 7 + 2 * j
        pm[T, 1] = rr * 7 + 2 * j + 1
    for pi in range(4):
        r0, r1 = 2 * pi, 2 * pi + 1
        pm[21 + pi, 0] = r0 * 7 + 6
        if r1 < 7:
            pm[21 + pi, 1] = r1 * 7 + 6
    return pm


def _prep_fc1(fc1):
    pm = _h2_posmap()
    fc1p = fc1.reshape(1024, 64, 49)
    fc1hat = np.zeros((1024, 25, 2, 64), np.float32)
    for T in range(25):
        for u in range(2):
            p = pm[T, u]
            if p >= 0:
                fc1hat[:, T, u, :] = fc1p[:, :, p]
    a = fc1hat.reshape(8, 128, 25, 128).transpose(3, 0, 2, 1)  # kp,m,k,mc
    return np.ascontiguousarray(a.reshape(128, 8 * 3200)).astype(np.float16)


def _prep_fc2(fc2):
    a = fc2.reshape(4, 128, 8, 128).transpose(3, 0, 2, 1)      # kp,m,k,mc
    return np.ascontiguousarray(a.reshape(128, 4 * 1024)).astype(np.float16)


def _prep_fc3(fc3):
    a = fc3.T.reshape(4, 128, 10).transpose(1, 0, 2)
    return np.ascontiguousarray(a.reshape(128, 40)).astype(np.float16)


# --------------------------------------------------------------- bass kernel

_NC_CACHE = []


def _build_nc():
    import concourse.bass as bass
    import concourse.mybir as mybir
    from concourse import bacc
    from concourse.tile import TileContext

    f32 = mybir.dt.float32
    f16 = mybir.dt.float16
    RELU = mybir.ActivationFunctionType.Relu

    nc = bacc.Bacc("TRN2", target_bir_lowering=False, debug=False,
                   num_devices=N_CORES)
    x_pp = nc.dram_tensor("x_pp", [128, 16 * 2 * BS], f16,
                          kind="ExternalInput")
    w1t = nc.dram_tensor("w1t", [128, 16 * 256], f16, kind="ExternalInput")
    w2p = nc.dram_tensor("w2p", [128, 21 * 1024], f16, kind="ExternalInput")
    w2s = nc.dram_tensor("w2s", [128, 7 * 512], f16, kind="ExternalInput")
    fc1m = nc.dram_tensor("fc1m", [128, 8 * 3200], f16, kind="ExternalInput")
    fc2t = nc.dram_tensor("fc2t", [128, 4 * 1024], f16, kind="ExternalInput")
    fc3t = nc.dram_tensor("fc3t", [128, 40], f16, kind="ExternalInput")
    y = nc.dram_tensor("y", [BS, 10], f32, kind="ExternalOutput")

    ectr = [0]

    with TileContext(nc) as tc:
        def relu_evac(dst, src):
            if ectr[0] % 2 == 0:
                nc.scalar.activation(dst, src, RELU)
            else:
                nc.vector.tensor_scalar_max(dst, src, 0.0)
            ectr[0] += 1

        with (
            tc.tile_pool(name="h2pool", bufs=25) as h2pool,
            tc.tile_pool(name="fcw", bufs=1) as fcw_pool,
            tc.tile_pool(name="scratch", bufs=1) as sc_pool,
        ):
            h2 = [h2pool.tile([128, 512], f16, tag="h2", name=f"h2_{T}")
                  for T in range(25)]
            ws = sc_pool.tile([128, 512], f16, tag="ws", name="ws")
            fc1wA = fcw_pool.tile([128, 4 * 3200], f16, tag="fc1wA",
                                  name="fc1wA")
            # --------------- phase 1: L1 + L2 interleaved ---------------
            with (
                tc.tile_pool(name="xp", bufs=1) as xpp_pool,
                tc.tile_pool(name="w1p", bufs=1) as w1_pool,
                tc.tile_pool(name="w2pp", bufs=3) as w2p_pool,
                tc.tile_pool(name="w2sp", bufs=1) as w2s_pool,
                tc.tile_pool(name="o1p", bufs=32) as o1_pool,
                tc.tile_pool(name="l1ps", bufs=2, space="PSUM") as l1ps,
                tc.tile_pool(name="l2ps", bufs=4, space="PSUM") as l2ps,
            ):
                # PE warmup with no DMA dependency: memset scratch, then
                # dummy MMs so HAM un-throttles while the first x chunk
                # streams in. K=128 full-row matmuls: HAM's activity
                # monitor does not register K=32 row-strip matmuls as
                # busy (observed: the clock stays at 4/8 until ~3.4us
                # after the first sustained full-row matmuls).
                nc.gpsimd.memset(ws[:], 0.0)
                nc.vector.memset(h2[24][64:128, :], 0.0)
                wps = l2ps.tile([128, 512], f32, tag="l2", name="warm_ps")

                def filler(n):
                    for wi in range(n):
                        nc.tensor.matmul(wps[:], ws[:, 0:128], ws[:],
                                         start=True, stop=True)

                filler(12)

                # resident x + w1; w2 main pairs stream per-pass
                xall = xpp_pool.tile([128, 16 * 1024], f16, tag="xa",
                                     name="xall")
                w1a = w1_pool.tile([128, 16 * 256], f16, tag="w1",
                                   name="w1a")
                w2sa = w2s_pool.tile([128, 7 * 512], f16, tag="w2s",
                                     name="w2sa")
                w2c = []

                def dma_w2c(h):
                    t = w2p_pool.tile([128, 3072], f16, tag="w2c",
                                      name=f"w2c_{h}")
                    nc.sync.dma_start(out=t[:],
                                      in_=w2p.ap()[:, 3072*h:3072*h+3072])
                    w2c.append(t)

                def dma_x(c0, c1):
                    nc.sync.dma_start(out=xall[:, c0:c1],
                                      in_=x_pp.ap()[:, c0:c1])

                # issue order = HBM service order (HWDGE FIFO): match the
                # compute pipeline's consumption order.
                nc.sync.dma_start(out=w1a[:, 0:1024],
                                  in_=w1t.ap()[:, 0:1024])
                for i in range(4):   # rows 0-1 split by PE row-strip
                    nc.sync.dma_start(out=xall[32*i:32*i+32, 0:2048],
                                      in_=x_pp.ap()[32*i:32*i+32, 0:2048])
                dma_x(2048, 3072)    # row 2
                dma_x(3072, 4096)    # row 3
                dma_w2c(0)
                dma_x(4096, 5120)    # row 4
                dma_x(5120, 6144)    # row 5
                dma_w2c(1)
                nc.sync.dma_start(out=w2sa[:], in_=w2s.ap())
                dma_x(6144, 7168)    # row 6
                dma_x(7168, 8192)    # row 7
                nc.sync.dma_start(out=w1a[:, 1024:4096],
                                  in_=w1t.ap()[:, 1024:4096])
                dma_w2c(2)
                dma_x(8192, 12288)   # rows 8-11
                dma_w2c(3)
                dma_x(12288, 16384)  # rows 12-15
                dma_w2c(4)
                nc.sync.dma_start(out=fc1wA[:, 0:6400],
                                  in_=fc1m.ap()[:, 0:6400])
                dma_w2c(5)
                dma_w2c(6)
                nc.sync.dma_start(out=fc1wA[:, 6400:12800],
                                  in_=fc1m.ap()[:, 6400:12800])

                out1d = [[None] * 4 for _ in range(16)]

                def out1(r, p):
                    u = p % 2
                    d = p // 2
                    base = 512 * u
                    return out1d[r][d][:, base:base + 512]

                def xsl(r, g, i):
                    c0 = 1024 * r + 512 * g
                    return xall[32*i:32*i+32, c0:c0 + 512]

                def w1sl(r, g, i):
                    c0 = r * 256 + g * 128
                    return w1a[32*i:32*i+32, c0:c0 + 128]

                def emit_l1_group(r, g):
                    pss = []
                    for d in range(2):
                        ps = l1ps.tile([128, 1024], f32, tag="l1",
                                       name=f"l1ps_{r}_{g}_{d}")
                        pss.append(ps)
                        for u in range(2):
                            i = 2 * d + u
                            nc.tensor.matmul(
                                ps[:, 512*u:512*u+512],
                                w1sl(r, g, i), xsl(r, g, i),
                                start=True, stop=True,
                                tile_position=(32 * i, 0))
                    for d in range(2):
                        ot = o1_pool.tile([128, 1024], f16, tag="o1",
                                          name=f"o1_{r}_{g}_{d}")
                        relu_evac(ot[:], pss[d][:])
                        out1d[r][2 * g + d] = ot

                def emit_main_pair(h, j):
                    T = 3 * h + j
                    wt = w2c[h][:, 1024*j:1024*j+1024]
                    ps = l2ps.tile([128, 512], f32, tag="l2",
                                   name=f"l2ps_{T}")
                    # combined full-width chain on the shared middle pair
                    for kh in range(4):
                        nc.tensor.matmul(
                            ps[:], wt[:, 128*kh:128*kh+128],
                            out1(2*h + kh, 2*j + 1),
                            start=(kh == 0), stop=False,
                            tile_position=(0, 0))
                    # outer pairs: A (left) on cols 0-63, B (right) on
                    # cols 64-127, interleaved for column-strip overlap
                    for kh in range(4):
                        nc.tensor.matmul(
                            ps[0:64, :], wt[:, 512+64*kh:512+64*kh+64],
                            out1(2*h + kh, 2*j),
                            start=False, stop=(kh == 3),
                            tile_position=(0, 0))
                        nc.tensor.matmul(
                            ps[64:128, :], wt[:, 768+64*kh:768+64*kh+64],
                            out1(2*h + kh, 2*j + 2),
                            start=False, stop=(kh == 3),
                            tile_position=(0, 64))
                    relu_evac(h2[T][:], ps[:])

                def emit_cross_pair(pi):
                    # positions (2pi, 6) and (2pi+1, 6) on column strips,
                    # separate PSUM banks (no shared rhs tile)
                    T = 21 + pi
                    hA, hB = 2 * pi, 2 * pi + 1
                    wA = w2sa[:, 512 * hA:512 * hA + 512]
                    wB = w2sa[:, 512 * hB:512 * hB + 512] if hB < 7 else None
                    psA = l2ps.tile([128, 512], f32, tag="l2",
                                    name=f"l2psA_{T}")
                    psB = None
                    if wB is not None:
                        psB = l2ps.tile([128, 512], f32, tag="l2",
                                        name=f"l2psB_{T}")
                    for kt in range(8):
                        kh, t = divmod(kt, 2)
                        nc.tensor.matmul(
                            psA[0:64, :], wA[:, 64*kt:64*kt+64],
                            out1(2*hA + kh, 6 + t),
                            start=(kt == 0), stop=(kt == 7),
                            tile_position=(0, 0))
                        if wB is not None:
                            nc.tensor.matmul(
                                psB[64:128, :], wB[:, 64*kt:64*kt+64],
                                out1(2*hB + kh, 6 + t),
                                start=(kt == 0), stop=(kt == 7),
                                tile_position=(0, 64))
                    relu_evac(h2[T][0:64, :], psA[0:64, :])
                    if wB is not None:
                        relu_evac(h2[T][64:128, :], psB[64:128, :])

                # lead-in: rows 0-3 (pass 0 depends on them); fillers
                # between groups keep the PE free of idle windows long
                # enough for HAM to re-throttle while the first DMAs land
                for r in range(4):
                    for g in range(2):
                        emit_l1_group(r, g)
                        filler(2)
                # steady state: interleave pass-h L2 chains with the L1
                # groups of rows 2h+4/2h+5 so every l1ps double's evac
                # hides under an L2 chain (l1ps has only 2 slots)
                for h in range(6):
                    chains = [lambda j=j: emit_main_pair(h, j)
                              for j in range(3)]
                    if h in (1, 3, 5):
                        chains.append(lambda: emit_cross_pair((h - 1) // 2))
                    groups = [(2*h + 4, 0), (2*h + 4, 1),
                              (2*h + 5, 0), (2*h + 5, 1)]
                    for k in range(4):
                        if k < len(chains):
                            chains[k]()
                        emit_l1_group(*groups[k])
                for j in range(3):
                    emit_main_pair(6, j)
                emit_cross_pair(3)

            # --------------- phase 2: FC head ---------------
            with (
                tc.tile_pool(name="fcio", bufs=12) as fcio_pool,
                tc.tile_pool(name="fcw2", bufs=1) as fcw2_pool,
                tc.tile_pool(name="fcps", bufs=2, space="PSUM") as fcps,
                tc.tile_pool(name="fc3ps", bufs=2, space="PSUM") as fc3ps,
            ):
                # second fc1 half + fc2/fc3 load into SBUF space freed by
                # the phase-1 pools, hidden under FC1 m0-m3 matmuls
                fc1wB = fcw2_pool.tile([128, 4 * 3200], f16, tag="fc1wB",
                                       name="fc1wB")
                nc.sync.dma_start(out=fc1wB[:, 0:6400],
                                  in_=fc1m.ap()[:, 12800:19200])
                nc.sync.dma_start(out=fc1wB[:, 6400:12800],
                                  in_=fc1m.ap()[:, 19200:25600])
                fc2w = fcw2_pool.tile([128, 4 * 1024], f16, tag="fc2w",
                                      name="fc2w")
                nc.sync.dma_start(out=fc2w[:], in_=fc2t.ap())
                w3 = fcw2_pool.tile([128, 40], f16, tag="fc3w", name="fc3w")
                nc.sync.dma_start(out=w3[:], in_=fc3t.ap())
                h3 = []
                for m in range(8):
                    wsrc = (fc1wA if m < 4 else fc1wB)
                    mo = 3200 * (m % 4)
                    ps = fcps.tile([128, 512], f32, tag="fc",
                                   name=f"fc1ps_{m}")
                    for k in range(25):
                        nc.tensor.matmul(
                            ps[:], wsrc[:, mo+128*k:mo+128*k+128],
                            h2[k][:],
                            start=(k == 0), stop=(k == 24))
                    ot = fcio_pool.tile([128, 512], f16, tag="h3",
                                        name=f"h3_{m}", bufs=8)
                    relu_evac(ot[:], ps[:])
                    h3.append(ot)
                h4 = []
                for m in range(4):
                    ps = fcps.tile([128, 512], f32, tag="fc",
                                   name=f"fc2ps_{m}")
                    for k in range(8):
                        nc.tensor.matmul(
                            ps[:],
                            fc2w[:, 1024*m+128*k:1024*m+128*k+128],
                            h3[k][:],
                            start=(k == 0), stop=(k == 7))
                    ot = fcio_pool.tile([128, 512], f16, tag="h4",
                                        name=f"h4_{m}", bufs=4)
                    relu_evac(ot[:], ps[:])
                    h4.append(ot)
                yt = fcio_pool.tile([128, 40], f32, tag="yout",
                                    name="yt", bufs=1)
                for b4 in range(4):
                    ps = fc3ps.tile([128, 10], f32, tag="fc3",
                                    name=f"fc3ps_{b4}")
                    for k in range(4):
                        nc.tensor.matmul(
                            ps[:],
                            h4[k][:, 128*b4:128*b4+128],
                            w3[:, 10*k:10*k+10],
                            start=(k == 0), stop=(k == 3))
                    nc.vector.tensor_copy(yt[:, 10*b4:10*b4+10], ps[:])
                nc.sync.dma_start(
                    out=y.ap().rearrange("(c p) o -> p c o", c=4),
                    in_=yt[:].rearrange("p (c o) -> p c o", c=4))
    nc.compile()
    return nc


def kernel(x, conv1w, conv2w, fc1, fc2, fc3):
    global LAST_EXEC_NS
    from concourse.bass_utils import run_bass_kernel_spmd

    x = np.ascontiguousarray(np.asarray(x, dtype=np.float32))
    conv1w = np.ascontiguousarray(np.asarray(conv1w, dtype=np.float32))
    conv2w = np.ascontiguousarray(np.asarray(conv2w, dtype=np.float32))
    fc1 = np.ascontiguousarray(np.asarray(fc1, dtype=np.float32))
    fc2 = np.ascontiguousarray(np.asarray(fc2, dtype=np.float32))
    fc3 = np.ascontiguousarray(np.asarray(fc3, dtype=np.float32))

    if not _NC_CACHE:
        _NC_CACHE.append(_build_nc())
    nc = _NC_CACHE[0]

    xpp = _prep_x(x)
    w2pm, w2sm = _prep_w2(conv2w)
    shared = {
        "w1t": _prep_w1(conv1w),
        "w2p": w2pm,
        "w2s": w2sm,
        "fc1m": _prep_fc1(fc1),
        "fc2t": _prep_fc2(fc2),
        "fc3t": _prep_fc3(fc3),
    }
    in_maps = [{**shared, "x_pp": xpp[c]} for c in range(N_CORES)]
    res = run_bass_kernel_spmd(nc, in_maps, list(range(N_CORES)))
    LAST_EXEC_NS = res.exec_time_ns
    return np.concatenate([r["y"] for r in res.results], axis=0)


# revision 14
# speedup vs baseline: 1.2971x; 1.0107x over previous
"""TRN2 Bass kernel for nn_CIFAR10_Type1_Template_Unroll (dense_cnn).

Network (per reference): two locally-connected conv layers + 3-layer FC
head, B=4096, fp32 in/out. Pure data parallel over 8 NeuronCores (512
batch each), activations on-chip in [feature, batch] layout, batch N=512
on the matmul free dim throughout. All DMA'd operands are fp16 (inputs
are O(1) normals; rounding is ~5e-4 relative, budget is 2e-2), halving
HBM traffic vs fp32.

DMA issue is serialized on the Sync engine at ~0.6us per dma_start, so
every input tensor is stored partition-major in DRAM and loaded with
~30 coarse transfers (0.25-1.6MB), ordered to match the consumption
order of the compute pipeline. fc1 weights are split: half prefetched
during phase 1, half loaded at phase-2 start into SBUF space freed by
the phase-1 pools, hidden under the first FC1 m-blocks' matmuls.

Layer mapping per core:
- L1 (k=2,s=2 locally-connected): host packs, per output row r and pair
  of adjacent positions, a K=32 strip (2 positions x 16 feats: 12 real +
  4 zero-pad) and a block-diagonal [32, 128] weight tile. 4 strips run
  concurrently on the PE's 32-row groups via tile_position=(32i, 0),
  writing two 2-bank PSUM doubles that are evacuated with single
  [128,1024] relu ops (evac cost scales with free dim only, so merging
  banks halves the per-bank cost; PSUM-source evacs run at 1x).
- L2 (k=4,s=2): positions are paired (h,2j)+(h,2j+1). The two positions
  share the middle input pair (2j+1), so a host-packed block weight
  [128,128] lets ONE full-width matmul per kh start the accumulation
  group for BOTH positions in one PSUM bank; the outer input pairs run
  as M=64 chains on PE column strips (0,0)/(0,64). One [128,512] relu
  evac per pair instead of two [64,512] halves. Column-7 positions pair
  across rows on column strips with separate banks (no shared rhs).
- FC head: K/M tiling, fc weights host-permuted to match the on-chip
  feature order of h2 ([pos-pair, parity, channel]).
A 16-matmul warmup on a memset scratch tile (no DMA dependency) ramps
the PE HAM clock gate from t~0 and bridges the first x-chunk DMA.
"""
import sys

if '/opt/trn_rl_repo' not in sys.path:
    sys.path.insert(0, '/opt/trn_rl_repo')

import numpy as np

N_CORES = 8
BS = 512
LAST_EXEC_NS = None

# ----------------------------------------------------------------- host prep

def _prep_x(x):
    """x [B,3,32,32] -> [N_CORES, 128, 16*2*512] fp16, partition-major.

    part = 32*i + 16*q + f; col = (r*2+g)*512 + b; pair p=4g+i covers
    w1 in {2p,2p+1}; q = w1 parity; f = c*4+kh*2+kw (12..15 zero-pad).
    """
    ncr = x.shape[0] // BS
    xr = x.reshape(ncr, BS, 3, 16, 2, 2, 4, 2, 2)   # s,b,c,r,kh,g,i,q,kw
    xt = xr.transpose(0, 3, 5, 6, 7, 2, 4, 8, 1)    # s,r,g,i,q,c,kh,kw,b
    xt = xt.reshape(ncr, 16, 2, 4, 2, 12, BS)
    xpp = np.zeros((ncr, 16, 2, 4, 2, 16, BS), np.float16)
    xpp[..., :12, :] = xt
    xpp = xpp.reshape(ncr, 16 * 2, 128, BS).transpose(0, 2, 1, 3)
    return np.ascontiguousarray(xpp.reshape(ncr, 128, 16 * 2 * BS))


def _prep_w1(conv1w):
    """conv1w [64,256,3,2,2] -> [128, 16*256] fp16 block-diag strips."""
    w1r = conv1w.reshape(64, 16, 16, 3, 2, 2)
    wt = w1r.transpose(1, 2, 3, 4, 5, 0).reshape(16, 16, 12, 64)
    wtp = np.zeros((16, 16, 16, 64), np.float32)
    wtp[:, :, :12, :] = wt
    wtp = wtp.reshape(16, 2, 4, 2, 16, 64)          # r,g,i,qp,f,o
    w1t = np.zeros((16, 2, 4, 2, 16, 2, 64), np.float32)
    w1t[:, :, :, 0, :, 0, :] = wtp[:, :, :, 0, :, :]
    w1t[:, :, :, 1, :, 1, :] = wtp[:, :, :, 1, :, :]
    w1t = w1t.reshape(16, 2, 128, 128).transpose(2, 0, 1, 3)  # p,r,g,c
    return np.ascontiguousarray(w1t.reshape(128, 16 * 256)).astype(np.float16)


def _prep_w2(conv2w):
    """conv2w [64,49,64,4,4] -> main [128,21*1024] + col-6 [128,7*512].

    Main pair T=(h, w'=2j / 2j+1): both positions read the middle input
    pair 2j+1, so cols 0:512 of pair block T hold, per kh, the
    [128,128] combined block (A's t=1 | B's t=0); cols 512:768 = A's
    left-pair (t=0) M=64 blocks; 768:1024 = B's right-pair (t=1).
    Partition = (q=kw%2)*64 + c, matching the L1 output tile layout.
    """
    wr = conv2w.reshape(64, 7, 7, 64, 4, 2, 2)      # o,h,w',c,kh,t,q
    wA = wr[:, :, 0:6:2]                            # o,h,j,c,kh,t,q
    wB = wr[:, :, 1:6:2]
    amid = wA[:, :, :, :, :, 1, :].transpose(1, 2, 5, 3, 4, 0)
    aleft = wA[:, :, :, :, :, 0, :].transpose(1, 2, 5, 3, 4, 0)
    bmid = wB[:, :, :, :, :, 0, :].transpose(1, 2, 5, 3, 4, 0)
    bright = wB[:, :, :, :, :, 1, :].transpose(1, 2, 5, 3, 4, 0)
    comb = np.stack([amid, bmid], axis=5)           # h,j,q,c,kh,half,o
    comb = comb.reshape(7, 3, 2, 64, 512)
    w2p = np.concatenate(
        [comb, aleft.reshape(7, 3, 2, 64, 256),
         bright.reshape(7, 3, 2, 64, 256)], axis=-1)   # h,j,q,c,1024
    w2p = w2p.transpose(2, 3, 0, 1, 4).reshape(128, 21 * 1024)
    w6 = wr[:, :, 6]                                # o,h,c,kh,t,q
    w2s = w6.transpose(5, 2, 1, 3, 4, 0).reshape(128, 7 * 512)
    return (np.ascontiguousarray(w2p).astype(np.float16),
            np.ascontiguousarray(w2s).astype(np.float16))


def _h2_posmap():
    pm = np.full((25, 2), -1, np.int64)
    for T in range(21):
        rr, j = divmod(T, 3)
        pm[T, 0] = rr * 7 + 2 * j
        pm[T, 1] = rr * 7 + 2 * j + 1
    for pi in range(4):
        r0, r1 = 2 * pi, 2 * pi + 1
        pm[21 + pi, 0] = r0 * 7 + 6
        if r1 < 7:
            pm[21 + pi, 1] = r1 * 7 + 6
    return pm


def _prep_fc1(fc1):
    pm = _h2_posmap()
    fc1p = fc1.reshape(1024, 64, 49)
    fc1hat = np.zeros((1024, 25, 2, 64), np.float32)
    for T in range(25):
        for u in range(2):
            p = pm[T, u]
            if p >= 0:
                fc1hat[:, T, u, :] = fc1p[:, :, p]
    a = fc1hat.reshape(8, 128, 25, 128).transpose(3, 0, 2, 1)  # kp,m,k,mc
    return np.ascontiguousarray(a.reshape(128, 8 * 3200)).astype(np.float16)


def _prep_fc2(fc2):
    a = fc2.reshape(4, 128, 8, 128).transpose(3, 0, 2, 1)      # kp,m,k,mc
    return np.ascontiguousarray(a.reshape(128, 4 * 1024)).astype(np.float16)


def _prep_fc3(fc3):
    a = fc3.T.reshape(4, 128, 10).transpose(1, 0, 2)
    return np.ascontiguousarray(a.reshape(128, 40)).astype(np.float16)


# --------------------------------------------------------------- bass kernel

_NC_CACHE = []


def _build_nc():
    import concourse.bass as bass
    import concourse.mybir as mybir
    from concourse import bacc
    from concourse.tile import TileContext

    f32 = mybir.dt.float32
    f16 = mybir.dt.float16
    RELU = mybir.ActivationFunctionType.Relu

    nc = bacc.Bacc("TRN2", target_bir_lowering=False, debug=False,
                   num_devices=N_CORES)
    x_pp = nc.dram_tensor("x_pp", [128, 16 * 2 * BS], f16,
                          kind="ExternalInput")
    w1t = nc.dram_tensor("w1t", [128, 16 * 256], f16, kind="ExternalInput")
    w2p = nc.dram_tensor("w2p", [128, 21 * 1024], f16, kind="ExternalInput")
    w2s = nc.dram_tensor("w2s", [128, 7 * 512], f16, kind="ExternalInput")
    fc1m = nc.dram_tensor("fc1m", [128, 8 * 3200], f16, kind="ExternalInput")
    fc2t = nc.dram_tensor("fc2t", [128, 4 * 1024], f16, kind="ExternalInput")
    fc3t = nc.dram_tensor("fc3t", [128, 40], f16, kind="ExternalInput")
    y = nc.dram_tensor("y", [BS, 10], f32, kind="ExternalOutput")

    ectr = [0]

    with TileContext(nc) as tc:
        def relu_evac(dst, src):
            if ectr[0] % 2 == 0:
                nc.scalar.activation(dst, src, RELU)
            else:
                nc.vector.tensor_scalar_max(dst, src, 0.0)
            ectr[0] += 1

        with (
            tc.tile_pool(name="h2pool", bufs=25) as h2pool,
            tc.tile_pool(name="fcw", bufs=1) as fcw_pool,
            tc.tile_pool(name="scratch", bufs=1) as sc_pool,
        ):
            h2 = [h2pool.tile([128, 512], f16, tag="h2", name=f"h2_{T}")
                  for T in range(25)]
            ws = sc_pool.tile([128, 512], f16, tag="ws", name="ws")
            fc1wA = fcw_pool.tile([128, 4 * 3200], f16, tag="fc1wA",
                                  name="fc1wA")
            # --------------- phase 1: L1 + L2 interleaved ---------------
            with (
                tc.tile_pool(name="xp", bufs=1) as xpp_pool,
                tc.tile_pool(name="w1p", bufs=1) as w1_pool,
                tc.tile_pool(name="w2pp", bufs=3) as w2p_pool,
                tc.tile_pool(name="w2sp", bufs=1) as w2s_pool,
                tc.tile_pool(name="o1p", bufs=32) as o1_pool,
                tc.tile_pool(name="l1ps", bufs=2, space="PSUM") as l1ps,
                tc.tile_pool(name="l2ps", bufs=4, space="PSUM") as l2ps,
            ):
                # PE warmup with no DMA dependency: memset scratch, then
                # dummy MMs so HAM un-throttles while the first x chunk
                # streams in. K=128 full-row matmuls: HAM's activity
                # monitor does not register K=32 row-strip matmuls as
                # busy (observed: the clock stays at 4/8 until ~3.4us
                # after the first sustained full-row matmuls).
                nc.gpsimd.memset(ws[:], 0.0)
                nc.vector.memset(h2[24][64:128, :], 0.0)
                wps = l2ps.tile([128, 512], f32, tag="l2", name="warm_ps")

                def filler(n):
                    for wi in range(n):
                        nc.tensor.matmul(wps[:], ws[:, 0:128], ws[:],
                                         start=True, stop=True)

                # sized to bridge until the first x/w1/w2 chunks land
                # (~15us): fillers behind a stalled real matmul never
                # execute (in-order queue), so the bridge must be an
                # uninterrupted prefix.
                filler(24)

                # resident x + w1; w2 main pairs stream per-pass
                xall = xpp_pool.tile([128, 16 * 1024], f16, tag="xa",
                                     name="xall")
                w1a = w1_pool.tile([128, 16 * 256], f16, tag="w1",
                                   name="w1a")
                w2sa = w2s_pool.tile([128, 7 * 512], f16, tag="w2s",
                                     name="w2sa")
                w2c = []

                def dma_w2c(h):
                    t = w2p_pool.tile([128, 3072], f16, tag="w2c",
                                      name=f"w2c_{h}")
                    nc.sync.dma_start(out=t[:],
                                      in_=w2p.ap()[:, 3072*h:3072*h+3072])
                    w2c.append(t)

                def dma_x(c0, c1):
                    nc.sync.dma_start(out=xall[:, c0:c1],
                                      in_=x_pp.ap()[:, c0:c1])

                # issue order = HBM service order (HWDGE FIFO): match the
                # compute pipeline's consumption order.
                nc.sync.dma_start(out=w1a[:, 0:1024],
                                  in_=w1t.ap()[:, 0:1024])
                for i in range(4):   # rows 0-1 split by PE row-strip
                    nc.sync.dma_start(out=xall[32*i:32*i+32, 0:2048],
                                      in_=x_pp.ap()[32*i:32*i+32, 0:2048])
                dma_x(2048, 3072)    # row 2
                dma_x(3072, 4096)    # row 3
                dma_w2c(0)
                dma_x(4096, 5120)    # row 4
                dma_x(5120, 6144)    # row 5
                dma_w2c(1)
                nc.sync.dma_start(out=w2sa[:], in_=w2s.ap())
                dma_x(6144, 7168)    # row 6
                dma_x(7168, 8192)    # row 7
                nc.sync.dma_start(out=w1a[:, 1024:4096],
                                  in_=w1t.ap()[:, 1024:4096])
                dma_w2c(2)
                dma_x(8192, 12288)   # rows 8-11
                dma_w2c(3)
                dma_x(12288, 16384)  # rows 12-15
                dma_w2c(4)
                nc.sync.dma_start(out=fc1wA[:, 0:6400],
                                  in_=fc1m.ap()[:, 0:6400])
                dma_w2c(5)
                dma_w2c(6)
                nc.sync.dma_start(out=fc1wA[:, 6400:12800],
                                  in_=fc1m.ap()[:, 6400:12800])

                out1d = [[None] * 4 for _ in range(16)]

                def out1(r, p):
                    u = p % 2
                    d = p // 2
                    base = 512 * u
                    return out1d[r][d][:, base:base + 512]

                def xsl(r, g, i):
                    c0 = 1024 * r + 512 * g
                    return xall[32*i:32*i+32, c0:c0 + 512]

                def w1sl(r, g, i):
                    c0 = r * 256 + g * 128
                    return w1a[32*i:32*i+32, c0:c0 + 128]

                def emit_l1_group(r, g):
                    pss = []
                    for d in range(2):
                        ps = l1ps.tile([128, 1024], f32, tag="l1",
                                       name=f"l1ps_{r}_{g}_{d}")
                        pss.append(ps)
                        for u in range(2):
                            i = 2 * d + u
                            nc.tensor.matmul(
                                ps[:, 512*u:512*u+512],
                                w1sl(r, g, i), xsl(r, g, i),
                                start=True, stop=True,
                                tile_position=(32 * i, 0))
                    for d in range(2):
                        ot = o1_pool.tile([128, 1024], f16, tag="o1",
                                          name=f"o1_{r}_{g}_{d}")
                        relu_evac(ot[:], pss[d][:])
                        out1d[r][2 * g + d] = ot

                def emit_main_pair(h, j):
                    T = 3 * h + j
                    wt = w2c[h][:, 1024*j:1024*j+1024]
                    ps = l2ps.tile([128, 512], f32, tag="l2",
                                   name=f"l2ps_{T}")
                    # combined full-width chain on the shared middle pair
                    for kh in range(4):
                        nc.tensor.matmul(
                            ps[:], wt[:, 128*kh:128*kh+128],
                            out1(2*h + kh, 2*j + 1),
                            start=(kh == 0), stop=False,
                            tile_position=(0, 0))
                    # outer pairs: A (left) on cols 0-63, B (right) on
                    # cols 64-127, interleaved for column-strip overlap
                    for kh in range(4):
                        nc.tensor.matmul(
                            ps[0:64, :], wt[:, 512+64*kh:512+64*kh+64],
                            out1(2*h + kh, 2*j),
                            start=False, stop=(kh == 3),
                            tile_position=(0, 0))
                        nc.tensor.matmul(
                            ps[64:128, :], wt[:, 768+64*kh:768+64*kh+64],
                            out1(2*h + kh, 2*j + 2),
                            start=False, stop=(kh == 3),
                            tile_position=(0, 64))
                    relu_evac(h2[T][:], ps[:])

                def emit_cross_pair(pi):
                    # positions (2pi, 6) and (2pi+1, 6) on column strips,
                    # separate PSUM banks (no shared rhs tile)
                    T = 21 + pi
                    hA, hB = 2 * pi, 2 * pi + 1
                    wA = w2sa[:, 512 * hA:512 * hA + 512]
                    wB = w2sa[:, 512 * hB:512 * hB + 512] if hB < 7 else None
                    psA = l2ps.tile([128, 512], f32, tag="l2",
                                    name=f"l2psA_{T}")
                    psB = None
                    if wB is not None:
                        psB = l2ps.tile([128, 512], f32, tag="l2",
                                        name=f"l2psB_{T}")
                    for kt in range(8):
                        kh, t = divmod(kt, 2)
                        nc.tensor.matmul(
                            psA[0:64, :], wA[:, 64*kt:64*kt+64],
                            out1(2*hA + kh, 6 + t),
                            start=(kt == 0), stop=(kt == 7),
                            tile_position=(0, 0))
                        if wB is not None:
                            nc.tensor.matmul(
                                psB[64:128, :], wB[:, 64*kt:64*kt+64],
                                out1(2*hB + kh, 6 + t),
                                start=(kt == 0), stop=(kt == 7),
                                tile_position=(0, 64))
                    relu_evac(h2[T][0:64, :], psA[0:64, :])
                    if wB is not None:
                        relu_evac(h2[T][64:128, :], psB[64:128, :])

                # lead-in: rows 0-3 (pass 0 depends on them); fillers
                # between groups keep the PE free of idle windows long
                # enough for HAM to re-throttle while the first DMAs land
                for r in range(4):
                    for g in range(2):
                        emit_l1_group(r, g)
                        filler(2)
                # steady state: interleave pass-h L2 chains with the L1
                # groups of rows 2h+4/2h+5 so every l1ps double's evac
                # hides under an L2 chain (l1ps has only 2 slots)
                for h in range(6):
                    chains = [lambda j=j: emit_main_pair(h, j)
                              for j in range(3)]
                    if h in (1, 3, 5):
                        chains.append(lambda: emit_cross_pair((h - 1) // 2))
                    groups = [(2*h + 4, 0), (2*h + 4, 1),
                              (2*h + 5, 0), (2*h + 5, 1)]
                    for k in range(4):
                        if k < len(chains):
                            chains[k]()
                        emit_l1_group(*groups[k])
                for j in range(3):
                    emit_main_pair(6, j)
                emit_cross_pair(3)

            # --------------- phase 2: FC head ---------------
            with (
                tc.tile_pool(name="fcio", bufs=12) as fcio_pool,
                tc.tile_pool(name="fcw2", bufs=1) as fcw2_pool,
                tc.tile_pool(name="fcps", bufs=2, space="PSUM") as fcps,
                tc.tile_pool(name="fc3ps", bufs=2, space="PSUM") as fc3ps,
            ):
                # second fc1 half + fc2/fc3 load into SBUF space freed by
                # the phase-1 pools, hidden under FC1 m0-m3 matmuls
                fc1wB = fcw2_pool.tile([128, 4 * 3200], f16, tag="fc1wB",
                                       name="fc1wB")
                nc.sync.dma_start(out=fc1wB[:, 0:6400],
                                  in_=fc1m.ap()[:, 12800:19200])
                nc.sync.dma_start(out=fc1wB[:, 6400:12800],
                                  in_=fc1m.ap()[:, 19200:25600])
                fc2w = fcw2_pool.tile([128, 4 * 1024], f16, tag="fc2w",
                                      name="fc2w")
                nc.sync.dma_start(out=fc2w[:], in_=fc2t.ap())
                w3 = fcw2_pool.tile([128, 40], f16, tag="fc3w", name="fc3w")
                nc.sync.dma_start(out=w3[:], in_=fc3t.ap())
                h3 = []
                for m in range(8):
                    wsrc = (fc1wA if m < 4 else fc1wB)
                    mo = 3200 * (m % 4)
                    ps = fcps.tile([128, 512], f32, tag="fc",
                                   name=f"fc1ps_{m}")
                    for k in range(25):
                        nc.tensor.matmul(
                            ps[:], wsrc[:, mo+128*k:mo+128*k+128],
                            h2[k][:],
                            start=(k == 0), stop=(k == 24))
                    ot = fcio_pool.tile([128, 512], f16, tag="h3",
                                        name=f"h3_{m}", bufs=8)
                    relu_evac(ot[:], ps[:])
                    h3.append(ot)
                h4 = []
                for m in range(4):
                    ps = fcps.tile([128, 512], f32, tag="fc",
                                   name=f"fc2ps_{m}")
                    for k in range(8):
                        nc.tensor.matmul(
                            ps[:],
                            fc2w[:, 1024*m+128*k:1024*m+128*k+128],
                            h3[k][:],
                            start=(k == 0), stop=(k == 7))
                    ot = fcio_pool.tile([128, 512], f16, tag="h4",
                                        name=f"h4_{m}", bufs=4)
                    relu_evac(ot[:], ps[:])
                    h4.append(ot)
                yt = fcio_pool.tile([128, 40], f32, tag="yout",
                                    name="yt", bufs=1)
                for b4 in range(4):
                    ps = fc3ps.tile([128, 10], f32, tag="fc3",
                                    name=f"fc3ps_{b4}")
                    for k in range(4):
                        nc.tensor.matmul(
                            ps[:],
                            h4[k][:, 128*b4:128*b4+128],
                            w3[:, 10*k:10*k+10],
                            start=(k == 0), stop=(k == 3))
                    nc.vector.tensor_copy(yt[:, 10*b4:10*b4+10], ps[:])
                nc.sync.dma_start(
                    out=y.ap().rearrange("(c p) o -> p c o", c=4),
                    in_=yt[:].rearrange("p (c o) -> p c o", c=4))
    nc.compile()
    return nc


def kernel(x, conv1w, conv2w, fc1, fc2, fc3):
    global LAST_EXEC_NS
    from concourse.bass_utils import run_bass_kernel_spmd

    x = np.ascontiguousarray(np.asarray(x, dtype=np.float32))
    conv1w = np.ascontiguousarray(np.asarray(conv1w, dtype=np.float32))
    conv2w = np.ascontiguousarray(np.asarray(conv2w, dtype=np.float32))
    fc1 = np.ascontiguousarray(np.asarray(fc1, dtype=np.float32))
    fc2 = np.ascontiguousarray(np.asarray(fc2, dtype=np.float32))
    fc3 = np.ascontiguousarray(np.asarray(fc3, dtype=np.float32))

    if not _NC_CACHE:
        _NC_CACHE.append(_build_nc())
    nc = _NC_CACHE[0]

    xpp = _prep_x(x)
    w2pm, w2sm = _prep_w2(conv2w)
    shared = {
        "w1t": _prep_w1(conv1w),
        "w2p": w2pm,
        "w2s": w2sm,
        "fc1m": _prep_fc1(fc1),
        "fc2t": _prep_fc2(fc2),
        "fc3t": _prep_fc3(fc3),
    }
    in_maps = [{**shared, "x_pp": xpp[c]} for c in range(N_CORES)]
    res = run_bass_kernel_spmd(nc, in_maps, list(range(N_CORES)))
    LAST_EXEC_NS = res.exec_time_ns
    return np.concatenate([r["y"] for r in res.results], axis=0)


# revision 15
# speedup vs baseline: 1.3056x; 1.0066x over previous
"""TRN2 Bass kernel for nn_CIFAR10_Type1_Template_Unroll (dense_cnn).

Network (per reference): two locally-connected conv layers + 3-layer FC
head, B=4096, fp32 in/out. Pure data parallel over 8 NeuronCores (512
batch each), activations on-chip in [feature, batch] layout, batch N=512
on the matmul free dim throughout. All DMA'd operands are fp16 (inputs
are O(1) normals; rounding is ~5e-4 relative, budget is 2e-2), halving
HBM traffic vs fp32.

DMA issue is serialized on the Sync engine at ~0.6us per dma_start, so
every input tensor is stored partition-major in DRAM and loaded with
~30 coarse transfers (0.25-1.6MB), ordered to match the consumption
order of the compute pipeline. fc1 weights are split: half prefetched
during phase 1, half loaded at phase-2 start into SBUF space freed by
the phase-1 pools, hidden under the first FC1 m-blocks' matmuls.

Layer mapping per core:
- L1 (k=2,s=2 locally-connected): host packs, per output row r and pair
  of adjacent positions, a K=32 strip (2 positions x 16 feats: 12 real +
  4 zero-pad) and a block-diagonal [32, 128] weight tile. 4 strips run
  concurrently on the PE's 32-row groups via tile_position=(32i, 0),
  writing two 2-bank PSUM doubles that are evacuated with single
  [128,1024] relu ops (evac cost scales with free dim only, so merging
  banks halves the per-bank cost; PSUM-source evacs run at 1x).
- L2 (k=4,s=2): positions are paired (h,2j)+(h,2j+1). The two positions
  share the middle input pair (2j+1), so a host-packed block weight
  [128,128] lets ONE full-width matmul per kh start the accumulation
  group for BOTH positions in one PSUM bank; the outer input pairs run
  as M=64 chains on PE column strips (0,0)/(0,64). One [128,512] relu
  evac per pair instead of two [64,512] halves. Column-7 positions pair
  across rows on column strips with separate banks (no shared rhs).
- FC head: K/M tiling, fc weights host-permuted to match the on-chip
  feature order of h2 ([pos-pair, parity, channel]).
A 16-matmul warmup on a memset scratch tile (no DMA dependency) ramps
the PE HAM clock gate from t~0 and bridges the first x-chunk DMA.
"""
import sys

if '/opt/trn_rl_repo' not in sys.path:
    sys.path.insert(0, '/opt/trn_rl_repo')

import numpy as np

N_CORES = 8
BS = 512
LAST_EXEC_NS = None

# ----------------------------------------------------------------- host prep

def _prep_x(x):
    """x [B,3,32,32] -> [N_CORES, 128, 16*2*512] fp16, partition-major.

    part = 32*i + 16*q + f; col = (r*2+g)*512 + b; pair p=4g+i covers
    w1 in {2p,2p+1}; q = w1 parity; f = c*4+kh*2+kw (12..15 zero-pad).
    """
    ncr = x.shape[0] // BS
    xr = x.reshape(ncr, BS, 3, 16, 2, 2, 4, 2, 2)   # s,b,c,r,kh,g,i,q,kw
    xt = xr.transpose(0, 3, 5, 6, 7, 2, 4, 8, 1)    # s,r,g,i,q,c,kh,kw,b
    xt = xt.reshape(ncr, 16, 2, 4, 2, 12, BS)
    xpp = np.zeros((ncr, 16, 2, 4, 2, 16, BS), np.float16)
    xpp[..., :12, :] = xt
    xpp = xpp.reshape(ncr, 16 * 2, 128, BS).transpose(0, 2, 1, 3)
    return np.ascontiguousarray(xpp.reshape(ncr, 128, 16 * 2 * BS))


def _prep_w1(conv1w):
    """conv1w [64,256,3,2,2] -> [128, 16*256] fp16 block-diag strips."""
    w1r = conv1w.reshape(64, 16, 16, 3, 2, 2)
    wt = w1r.transpose(1, 2, 3, 4, 5, 0).reshape(16, 16, 12, 64)
    wtp = np.zeros((16, 16, 16, 64), np.float32)
    wtp[:, :, :12, :] = wt
    wtp = wtp.reshape(16, 2, 4, 2, 16, 64)          # r,g,i,qp,f,o
    w1t = np.zeros((16, 2, 4, 2, 16, 2, 64), np.float32)
    w1t[:, :, :, 0, :, 0, :] = wtp[:, :, :, 0, :, :]
    w1t[:, :, :, 1, :, 1, :] = wtp[:, :, :, 1, :, :]
    w1t = w1t.reshape(16, 2, 128, 128).transpose(2, 0, 1, 3)  # p,r,g,c
    return np.ascontiguousarray(w1t.reshape(128, 16 * 256)).astype(np.float16)


def _prep_w2(conv2w):
    """conv2w [64,49,64,4,4] -> main [128,21*1024] + col-6 [128,7*512].

    Main pair T=(h, w'=2j / 2j+1): both positions read the middle input
    pair 2j+1, so cols 0:512 of pair block T hold, per kh, the
    [128,128] combined block (A's t=1 | B's t=0); cols 512:768 = A's
    left-pair (t=0) M=64 blocks; 768:1024 = B's right-pair (t=1).
    Partition = (q=kw%2)*64 + c, matching the L1 output tile layout.
    """
    wr = conv2w.reshape(64, 7, 7, 64, 4, 2, 2)      # o,h,w',c,kh,t,q
    wA = wr[:, :, 0:6:2]                            # o,h,j,c,kh,t,q
    wB = wr[:, :, 1:6:2]
    amid = wA[:, :, :, :, :, 1, :].transpose(1, 2, 5, 3, 4, 0)
    aleft = wA[:, :, :, :, :, 0, :].transpose(1, 2, 5, 3, 4, 0)
    bmid = wB[:, :, :, :, :, 0, :].transpose(1, 2, 5, 3, 4, 0)
    bright = wB[:, :, :, :, :, 1, :].transpose(1, 2, 5, 3, 4, 0)
    comb = np.stack([amid, bmid], axis=5)           # h,j,q,c,kh,half,o
    comb = comb.reshape(7, 3, 2, 64, 512)
    w2p = np.concatenate(
        [comb, aleft.reshape(7, 3, 2, 64, 256),
         bright.reshape(7, 3, 2, 64, 256)], axis=-1)   # h,j,q,c,1024
    w2p = w2p.transpose(2, 3, 0, 1, 4).reshape(128, 21 * 1024)
    w6 = wr[:, :, 6]                                # o,h,c,kh,t,q
    w2s = w6.transpose(5, 2, 1, 3, 4, 0).reshape(128, 7 * 512)
    return (np.ascontiguousarray(w2p).astype(np.float16),
            np.ascontiguousarray(w2s).astype(np.float16))


def _h2_posmap():
    pm = np.full((25, 2), -1, np.int64)
    for T in range(21):
        rr, j = divmod(T, 3)
        pm[T, 0] = rr * 7 + 2 * j
        pm[T, 1] = rr * 7 + 2 * j + 1
    for pi in range(4):
        r0, r1 = 2 * pi, 2 * pi + 1
        pm[21 + pi, 0] = r0 * 7 + 6
        if r1 < 7:
            pm[21 + pi, 1] = r1 * 7 + 6
    return pm


def _prep_fc1(fc1):
    pm = _h2_posmap()
    fc1p = fc1.reshape(1024, 64, 49)
    fc1hat = np.zeros((1024, 25, 2, 64), np.float32)
    for T in range(25):
        for u in range(2):
            p = pm[T, u]
            if p >= 0:
                fc1hat[:, T, u, :] = fc1p[:, :, p]
    a = fc1hat.reshape(8, 128, 25, 128).transpose(3, 0, 2, 1)  # kp,m,k,mc
    return np.ascontiguousarray(a.reshape(128, 8 * 3200)).astype(np.float16)


def _prep_fc2(fc2):
    a = fc2.reshape(4, 128, 8, 128).transpose(3, 0, 2, 1)      # kp,m,k,mc
    return np.ascontiguousarray(a.reshape(128, 4 * 1024)).astype(np.float16)


def _prep_fc3(fc3):
    a = fc3.T.reshape(4, 128, 10).transpose(1, 0, 2)
    return np.ascontiguousarray(a.reshape(128, 40)).astype(np.float16)


# --------------------------------------------------------------- bass kernel

_NC_CACHE = []


def _build_nc():
    import concourse.bass as bass
    import concourse.mybir as mybir
    from concourse import bacc
    from concourse.tile import TileContext

    f32 = mybir.dt.float32
    f16 = mybir.dt.float16
    RELU = mybir.ActivationFunctionType.Relu

    nc = bacc.Bacc("TRN2", target_bir_lowering=False, debug=False,
                   num_devices=N_CORES)
    x_pp = nc.dram_tensor("x_pp", [128, 16 * 2 * BS], f16,
                          kind="ExternalInput")
    w1t = nc.dram_tensor("w1t", [128, 16 * 256], f16, kind="ExternalInput")
    w2p = nc.dram_tensor("w2p", [128, 21 * 1024], f16, kind="ExternalInput")
    w2s = nc.dram_tensor("w2s", [128, 7 * 512], f16, kind="ExternalInput")
    fc1m = nc.dram_tensor("fc1m", [128, 8 * 3200], f16, kind="ExternalInput")
    fc2t = nc.dram_tensor("fc2t", [128, 4 * 1024], f16, kind="ExternalInput")
    fc3t = nc.dram_tensor("fc3t", [128, 40], f16, kind="ExternalInput")
    y = nc.dram_tensor("y", [BS, 10], f32, kind="ExternalOutput")

    ectr = [0]

    with TileContext(nc) as tc:
        def relu_evac(dst, src):
            if ectr[0] % 2 == 0:
                nc.scalar.activation(dst, src, RELU)
            else:
                nc.vector.tensor_scalar_max(dst, src, 0.0)
            ectr[0] += 1

        with (
            tc.tile_pool(name="h2pool", bufs=25) as h2pool,
            tc.tile_pool(name="fcw", bufs=1) as fcw_pool,
            tc.tile_pool(name="scratch", bufs=1) as sc_pool,
        ):
            h2 = [h2pool.tile([128, 512], f16, tag="h2", name=f"h2_{T}")
                  for T in range(25)]
            ws = sc_pool.tile([128, 512], f16, tag="ws", name="ws")
            fc1wA = fcw_pool.tile([128, 4 * 3200], f16, tag="fc1wA",
                                  name="fc1wA")
            # --------------- phase 1: L1 + L2 interleaved ---------------
            with (
                tc.tile_pool(name="xp", bufs=1) as xpp_pool,
                tc.tile_pool(name="w1p", bufs=1) as w1_pool,
                tc.tile_pool(name="w2pp", bufs=3) as w2p_pool,
                tc.tile_pool(name="w2sp", bufs=1) as w2s_pool,
                tc.tile_pool(name="o1p", bufs=32) as o1_pool,
                tc.tile_pool(name="l1ps", bufs=2, space="PSUM") as l1ps,
                tc.tile_pool(name="l2ps", bufs=4, space="PSUM") as l2ps,
            ):
                # PE warmup with no DMA dependency: memset scratch, then
                # dummy MMs so HAM un-throttles while the first x chunk
                # streams in. K=128 full-row matmuls: HAM's activity
                # monitor does not register K=32 row-strip matmuls as
                # busy (observed: the clock stays at 4/8 until ~3.4us
                # after the first sustained full-row matmuls).
                nc.gpsimd.memset(ws[:], 0.0)
                nc.vector.memset(h2[24][64:128, :], 0.0)
                wps = l2ps.tile([128, 512], f32, tag="l2", name="warm_ps")

                def filler(n):
                    for wi in range(n):
                        nc.tensor.matmul(wps[:], ws[:, 0:128], ws[:],
                                         start=True, stop=True)

                # sized to bridge until the first x/w1/w2 chunks land
                # (~15us): fillers behind a stalled real matmul never
                # execute (in-order queue), so the bridge must be an
                # uninterrupted prefix.
                filler(24)

                # resident x + w1; w2 main pairs stream per-pass
                xall = xpp_pool.tile([128, 16 * 1024], f16, tag="xa",
                                     name="xall")
                w1a = w1_pool.tile([128, 16 * 256], f16, tag="w1",
                                   name="w1a")
                w2sa = w2s_pool.tile([128, 7 * 512], f16, tag="w2s",
                                     name="w2sa")
                w2c = []

                def dma_w2c(h):
                    t = w2p_pool.tile([128, 3072], f16, tag="w2c",
                                      name=f"w2c_{h}")
                    nc.sync.dma_start(out=t[:],
                                      in_=w2p.ap()[:, 3072*h:3072*h+3072])
                    w2c.append(t)

                def dma_x(c0, c1):
                    nc.sync.dma_start(out=xall[:, c0:c1],
                                      in_=x_pp.ap()[:, c0:c1])

                # issue order = HBM service order (HWDGE FIFO): match the
                # compute pipeline's consumption order.
                nc.sync.dma_start(out=w1a[:, 0:1536],   # rows 0-5
                                  in_=w1t.ap()[:, 0:1536])
                for i in range(4):   # rows 0-1 split by PE row-strip
                    nc.sync.dma_start(out=xall[32*i:32*i+32, 0:2048],
                                      in_=x_pp.ap()[32*i:32*i+32, 0:2048])
                dma_x(2048, 3072)    # row 2
                dma_x(3072, 4096)    # row 3
                dma_w2c(0)
                dma_x(4096, 5120)    # row 4
                dma_x(5120, 6144)    # row 5
                dma_w2c(1)
                dma_x(6144, 7168)    # row 6
                dma_x(7168, 8192)    # row 7
                nc.sync.dma_start(out=w1a[:, 1536:2560],  # rows 6-9
                                  in_=w1t.ap()[:, 1536:2560])
                nc.sync.dma_start(out=w2sa[:], in_=w2s.ap())
                dma_x(8192, 12288)   # rows 8-11
                dma_w2c(2)
                nc.sync.dma_start(out=w1a[:, 2560:4096],  # rows 10-15
                                  in_=w1t.ap()[:, 2560:4096])
                dma_w2c(3)
                dma_x(12288, 16384)  # rows 12-15
                dma_w2c(4)
                nc.sync.dma_start(out=fc1wA[:, 0:6400],
                                  in_=fc1m.ap()[:, 0:6400])
                dma_w2c(5)
                dma_w2c(6)
                nc.sync.dma_start(out=fc1wA[:, 6400:12800],
                                  in_=fc1m.ap()[:, 6400:12800])

                out1d = [[None] * 4 for _ in range(16)]

                def out1(r, p):
                    u = p % 2
                    d = p // 2
                    base = 512 * u
                    return out1d[r][d][:, base:base + 512]

                def xsl(r, g, i):
                    c0 = 1024 * r + 512 * g
                    return xall[32*i:32*i+32, c0:c0 + 512]

                def w1sl(r, g, i):
                    c0 = r * 256 + g * 128
                    return w1a[32*i:32*i+32, c0:c0 + 128]

                def emit_l1_group(r, g):
                    pss = []
                    for d in range(2):
                        ps = l1ps.tile([128, 1024], f32, tag="l1",
                                       name=f"l1ps_{r}_{g}_{d}")
                        pss.append(ps)
                        for u in range(2):
                            i = 2 * d + u
                            nc.tensor.matmul(
                                ps[:, 512*u:512*u+512],
                                w1sl(r, g, i), xsl(r, g, i),
                                start=True, stop=True,
                                tile_position=(32 * i, 0))
                    for d in range(2):
                        ot = o1_pool.tile([128, 1024], f16, tag="o1",
                                          name=f"o1_{r}_{g}_{d}")
                        relu_evac(ot[:], pss[d][:])
                        out1d[r][2 * g + d] = ot

                def emit_main_pair(h, j):
                    T = 3 * h + j
                    wt = w2c[h][:, 1024*j:1024*j+1024]
                    ps = l2ps.tile([128, 512], f32, tag="l2",
                                   name=f"l2ps_{T}")
                    # combined full-width chain on the shared middle pair
                    for kh in range(4):
                        nc.tensor.matmul(
                            ps[:], wt[:, 128*kh:128*kh+128],
                            out1(2*h + kh, 2*j + 1),
                            start=(kh == 0), stop=False,
                            tile_position=(0, 0))
                    # outer pairs: A (left) on cols 0-63, B (right) on
                    # cols 64-127, interleaved for column-strip overlap
                    for kh in range(4):
                        nc.tensor.matmul(
                            ps[0:64, :], wt[:, 512+64*kh:512+64*kh+64],
                            out1(2*h + kh, 2*j),
                            start=False, stop=(kh == 3),
                            tile_position=(0, 0))
                        nc.tensor.matmul(
                            ps[64:128, :], wt[:, 768+64*kh:768+64*kh+64],
                            out1(2*h + kh, 2*j + 2),
                            start=False, stop=(kh == 3),
                            tile_position=(0, 64))
                    relu_evac(h2[T][:], ps[:])

                def emit_cross_pair(pi):
                    # positions (2pi, 6) and (2pi+1, 6) on column strips,
                    # separate PSUM banks (no shared rhs tile)
                    T = 21 + pi
                    hA, hB = 2 * pi, 2 * pi + 1
                    wA = w2sa[:, 512 * hA:512 * hA + 512]
                    wB = w2sa[:, 512 * hB:512 * hB + 512] if hB < 7 else None
                    psA = l2ps.tile([128, 512], f32, tag="l2",
                                    name=f"l2psA_{T}")
                    psB = None
                    if wB is not None:
                        psB = l2ps.tile([128, 512], f32, tag="l2",
                                        name=f"l2psB_{T}")
                    for kt in range(8):
                        kh, t = divmod(kt, 2)
                        nc.tensor.matmul(
                            psA[0:64, :], wA[:, 64*kt:64*kt+64],
                            out1(2*hA + kh, 6 + t),
                            start=(kt == 0), stop=(kt == 7),
                            tile_position=(0, 0))
                        if wB is not None:
                            nc.tensor.matmul(
                                psB[64:128, :], wB[:, 64*kt:64*kt+64],
                                out1(2*hB + kh, 6 + t),
                                start=(kt == 0), stop=(kt == 7),
                                tile_position=(0, 64))
                    relu_evac(h2[T][0:64, :], psA[0:64, :])
                    if wB is not None:
                        relu_evac(h2[T][64:128, :], psB[64:128, :])

                # lead-in: rows 0-3 (pass 0 depends on them); fillers
                # between groups keep the PE free of idle windows long
                # enough for HAM to re-throttle while the first DMAs land
                for r in range(4):
                    for g in range(2):
                        emit_l1_group(r, g)
                        filler(2)
                # steady state: interleave pass-h L2 chains with the L1
                # groups of rows 2h+4/2h+5 so every l1ps double's evac
                # hides under an L2 chain (l1ps has only 2 slots)
                for h in range(6):
                    chains = [lambda j=j: emit_main_pair(h, j)
                              for j in range(3)]
                    if h in (1, 3, 5):
                        chains.append(lambda: emit_cross_pair((h - 1) // 2))
                    groups = [(2*h + 4, 0), (2*h + 4, 1),
                              (2*h + 5, 0), (2*h + 5, 1)]
                    for k in range(4):
                        if k < len(chains):
                            chains[k]()
                        emit_l1_group(*groups[k])
                for j in range(3):
                    emit_main_pair(6, j)
                emit_cross_pair(3)

            # --------------- phase 2: FC head ---------------
            with (
                tc.tile_pool(name="fcio", bufs=12) as fcio_pool,
                tc.tile_pool(name="fcw2", bufs=1) as fcw2_pool,
                tc.tile_pool(name="fcps", bufs=2, space="PSUM") as fcps,
                tc.tile_pool(name="fc3ps", bufs=2, space="PSUM") as fc3ps,
            ):
                # second fc1 half + fc2/fc3 load into SBUF space freed by
                # the phase-1 pools, hidden under FC1 m0-m3 matmuls
                fc1wB = fcw2_pool.tile([128, 4 * 3200], f16, tag="fc1wB",
                                       name="fc1wB")
                nc.sync.dma_start(out=fc1wB[:, 0:6400],
                                  in_=fc1m.ap()[:, 12800:19200])
                nc.sync.dma_start(out=fc1wB[:, 6400:12800],
                                  in_=fc1m.ap()[:, 19200:25600])
                fc2w = fcw2_pool.tile([128, 4 * 1024], f16, tag="fc2w",
                                      name="fc2w")
                nc.sync.dma_start(out=fc2w[:], in_=fc2t.ap())
                w3 = fcw2_pool.tile([128, 40], f16, tag="fc3w", name="fc3w")
                nc.sync.dma_start(out=w3[:], in_=fc3t.ap())
                h3 = []
                for m in range(8):
                    wsrc = (fc1wA if m < 4 else fc1wB)
                    mo = 3200 * (m % 4)
                    ps = fcps.tile([128, 512], f32, tag="fc",
                                   name=f"fc1ps_{m}")
                    for k in range(25):
                        nc.tensor.matmul(
                            ps[:], wsrc[:, mo+128*k:mo+128*k+128],
                            h2[k][:],
                            start=(k == 0), stop=(k == 24))
                    ot = fcio_pool.tile([128, 512], f16, tag="h3",
                                        name=f"h3_{m}", bufs=8)
                    relu_evac(ot[:], ps[:])
                    h3.append(ot)
                h4 = []
                for m in range(4):
                    ps = fcps.tile([128, 512], f32, tag="fc",
                                   name=f"fc2ps_{m}")
                    for k in range(8):
                        nc.tensor.matmul(
                            ps[:],
                            fc2w[:, 1024*m+128*k:1024*m+128*k+128],
                            h3[k][:],
                            start=(k == 0), stop=(k == 7))
                    ot = fcio_pool.tile([128, 512], f16, tag="h4",
                                        name=f"h4_{m}", bufs=4)
                    relu_evac(ot[:], ps[:])
                    h4.append(ot)
                yt = fcio_pool.tile([128, 40], f32, tag="yout",
                                    name="yt", bufs=1)
                for b4 in range(4):
                    ps = fc3ps.tile([128, 10], f32, tag="fc3",
                                    name=f"fc3ps_{b4}")
                    for k in range(4):
                        nc.tensor.matmul(
                            ps[:],
                            h4[k][:, 128*b4:128*b4+128],
                            w3[:, 10*k:10*k+10],
                            start=(k == 0), stop=(k == 3))
                    nc.vector.tensor_copy(yt[:, 10*b4:10*b4+10], ps[:])
                nc.sync.dma_start(
                    out=y.ap().rearrange("(c p) o -> p c o", c=4),
                    in_=yt[:].rearrange("p (c o) -> p c o", c=4))
    nc.compile()
    return nc


def kernel(x, conv1w, conv2w, fc1, fc2, fc3):
    global LAST_EXEC_NS
    from concourse.bass_utils import run_bass_kernel_spmd

    x = np.ascontiguousarray(np.asarray(x, dtype=np.float32))
    conv1w = np.ascontiguousarray(np.asarray(conv1w, dtype=np.float32))
    conv2w = np.ascontiguousarray(np.asarray(conv2w, dtype=np.float32))
    fc1 = np.ascontiguousarray(np.asarray(fc1, dtype=np.float32))
    fc2 = np.ascontiguousarray(np.asarray(fc2, dtype=np.float32))
    fc3 = np.ascontiguousarray(np.asarray(fc3, dtype=np.float32))

    if not _NC_CACHE:
        _NC_CACHE.append(_build_nc())
    nc = _NC_CACHE[0]

    xpp = _prep_x(x)
    w2pm, w2sm = _prep_w2(conv2w)
    shared = {
        "w1t": _prep_w1(conv1w),
        "w2p": w2pm,
        "w2s": w2sm,
        "fc1m": _prep_fc1(fc1),
        "fc2t": _prep_fc2(fc2),
        "fc3t": _prep_fc3(fc3),
    }
    in_maps = [{**shared, "x_pp": xpp[c]} for c in range(N_CORES)]
    res = run_bass_kernel_spmd(nc, in_maps, list(range(N_CORES)))
    LAST_EXEC_NS = res.exec_time_ns
    return np.concatenate([r["y"] for r in res.results], axis=0)


# revision 17
# speedup vs baseline: 1.3121x; 1.0050x over previous
"""TRN2 Bass kernel for nn_CIFAR10_Type1_Template_Unroll (dense_cnn).

Network (per reference): two locally-connected conv layers + 3-layer FC
head, B=4096, fp32 in/out. Pure data parallel over 8 NeuronCores (512
batch each), activations on-chip in [feature, batch] layout, batch N=512
on the matmul free dim throughout. All DMA'd operands are fp16 (inputs
are O(1) normals; rounding is ~5e-4 relative, budget is 2e-2), halving
HBM traffic vs fp32.

DMA issue is serialized on the Sync engine at ~0.6us per dma_start, so
every input tensor is stored partition-major in DRAM and loaded with
~30 coarse transfers (0.25-1.6MB), ordered to match the consumption
order of the compute pipeline. fc1 weights are split: half prefetched
during phase 1, half loaded at phase-2 start into SBUF space freed by
the phase-1 pools, hidden under the first FC1 m-blocks' matmuls.

Layer mapping per core:
- L1 (k=2,s=2 locally-connected): host packs, per output row r and pair
  of adjacent positions, a K=32 strip (2 positions x 16 feats: 12 real +
  4 zero-pad) and a block-diagonal [32, 128] weight tile. 4 strips run
  concurrently on the PE's 32-row groups via tile_position=(32i, 0),
  writing two 2-bank PSUM doubles that are evacuated with single
  [128,1024] relu ops (evac cost scales with free dim only, so merging
  banks halves the per-bank cost; PSUM-source evacs run at 1x).
- L2 (k=4,s=2): positions are paired (h,2j)+(h,2j+1). The two positions
  share the middle input pair (2j+1), so a host-packed block weight
  [128,128] lets ONE full-width matmul per kh start the accumulation
  group for BOTH positions in one PSUM bank; the outer input pairs run
  as M=64 chains on PE column strips (0,0)/(0,64). One [128,512] relu
  evac per pair instead of two [64,512] halves. Column-7 positions pair
  across rows on column strips with separate banks (no shared rhs).
- FC head: K/M tiling, fc weights host-permuted to match the on-chip
  feature order of h2 ([pos-pair, parity, channel]).
A 16-matmul warmup on a memset scratch tile (no DMA dependency) ramps
the PE HAM clock gate from t~0 and bridges the first x-chunk DMA.
"""
import sys

if '/opt/trn_rl_repo' not in sys.path:
    sys.path.insert(0, '/opt/trn_rl_repo')

import numpy as np

N_CORES = 8
BS = 512
LAST_EXEC_NS = None

# ----------------------------------------------------------------- host prep

def _prep_x(x):
    """x [B,3,32,32] -> [N_CORES, 128, 16*2*512] fp16, partition-major.

    part = 32*i + 16*q + f; col = (r*2+g)*512 + b; pair p=4g+i covers
    w1 in {2p,2p+1}; q = w1 parity; f = c*4+kh*2+kw (12..15 zero-pad).
    """
    ncr = x.shape[0] // BS
    xr = x.reshape(ncr, BS, 3, 16, 2, 2, 4, 2, 2)   # s,b,c,r,kh,g,i,q,kw
    xt = xr.transpose(0, 3, 5, 6, 7, 2, 4, 8, 1)    # s,r,g,i,q,c,kh,kw,b
    xt = xt.reshape(ncr, 16, 2, 4, 2, 12, BS)
    xpp = np.zeros((ncr, 16, 2, 4, 2, 16, BS), np.float16)
    xpp[..., :12, :] = xt
    xpp = xpp.reshape(ncr, 16 * 2, 128, BS).transpose(0, 2, 1, 3)
    return np.ascontiguousarray(xpp.reshape(ncr, 128, 16 * 2 * BS))


def _prep_w1(conv1w):
    """conv1w [64,256,3,2,2] -> [128, 16*256] fp16 block-diag strips."""
    w1r = conv1w.reshape(64, 16, 16, 3, 2, 2)
    wt = w1r.transpose(1, 2, 3, 4, 5, 0).reshape(16, 16, 12, 64)
    wtp = np.zeros((16, 16, 16, 64), np.float32)
    wtp[:, :, :12, :] = wt
    wtp = wtp.reshape(16, 2, 4, 2, 16, 64)          # r,g,i,qp,f,o
    w1t = np.zeros((16, 2, 4, 2, 16, 2, 64), np.float32)
    w1t[:, :, :, 0, :, 0, :] = wtp[:, :, :, 0, :, :]
    w1t[:, :, :, 1, :, 1, :] = wtp[:, :, :, 1, :, :]
    w1t = w1t.reshape(16, 2, 128, 128).transpose(2, 0, 1, 3)  # p,r,g,c
    return np.ascontiguousarray(w1t.reshape(128, 16 * 256)).astype(np.float16)


def _prep_w2(conv2w):
    """conv2w [64,49,64,4,4] -> main [128,21*1024] + col-6 [128,7*512].

    Main pair T=(h, w'=2j / 2j+1): both positions read the middle input
    pair 2j+1, so cols 0:512 of pair block T hold, per kh, the
    [128,128] combined block (A's t=1 | B's t=0); cols 512:768 = A's
    left-pair (t=0) M=64 blocks; 768:1024 = B's right-pair (t=1).
    Partition = (q=kw%2)*64 + c, matching the L1 output tile layout.
    """
    wr = conv2w.reshape(64, 7, 7, 64, 4, 2, 2)      # o,h,w',c,kh,t,q
    wA = wr[:, :, 0:6:2]                            # o,h,j,c,kh,t,q
    wB = wr[:, :, 1:6:2]
    amid = wA[:, :, :, :, :, 1, :].transpose(1, 2, 5, 3, 4, 0)
    aleft = wA[:, :, :, :, :, 0, :].transpose(1, 2, 5, 3, 4, 0)
    bmid = wB[:, :, :, :, :, 0, :].transpose(1, 2, 5, 3, 4, 0)
    bright = wB[:, :, :, :, :, 1, :].transpose(1, 2, 5, 3, 4, 0)
    comb = np.stack([amid, bmid], axis=5)           # h,j,q,c,kh,half,o
    comb = comb.reshape(7, 3, 2, 64, 512)
    w2p = np.concatenate(
        [comb, aleft.reshape(7, 3, 2, 64, 256),
         bright.reshape(7, 3, 2, 64, 256)], axis=-1)   # h,j,q,c,1024
    w2p = w2p.transpose(2, 3, 0, 1, 4).reshape(128, 21 * 1024)
    w6 = wr[:, :, 6]                                # o,h,c,kh,t,q
    w2s = w6.transpose(5, 2, 1, 3, 4, 0).reshape(128, 7 * 512)
    return (np.ascontiguousarray(w2p).astype(np.float16),
            np.ascontiguousarray(w2s).astype(np.float16))


def _h2_posmap():
    pm = np.full((25, 2), -1, np.int64)
    for T in range(21):
        rr, j = divmod(T, 3)
        pm[T, 0] = rr * 7 + 2 * j
        pm[T, 1] = rr * 7 + 2 * j + 1
    for pi in range(4):
        r0, r1 = 2 * pi, 2 * pi + 1
        pm[21 + pi, 0] = r0 * 7 + 6
        if r1 < 7:
            pm[21 + pi, 1] = r1 * 7 + 6
    return pm


def _prep_fc1(fc1):
    pm = _h2_posmap()
    fc1p = fc1.reshape(1024, 64, 49)
    fc1hat = np.zeros((1024, 25, 2, 64), np.float32)
    for T in range(25):
        for u in range(2):
            p = pm[T, u]
            if p >= 0:
                fc1hat[:, T, u, :] = fc1p[:, :, p]
    a = fc1hat.reshape(8, 128, 25, 128).transpose(3, 0, 2, 1)  # kp,m,k,mc
    return np.ascontiguousarray(a.reshape(128, 8 * 3200)).astype(np.float16)


def _prep_fc2(fc2):
    a = fc2.reshape(4, 128, 8, 128).transpose(3, 0, 2, 1)      # kp,m,k,mc
    return np.ascontiguousarray(a.reshape(128, 4 * 1024)).astype(np.float16)


def _prep_fc3(fc3):
    a = fc3.T.reshape(4, 128, 10).transpose(1, 0, 2)
    return np.ascontiguousarray(a.reshape(128, 40)).astype(np.float16)


# --------------------------------------------------------------- bass kernel

_NC_CACHE = []


def _build_nc():
    import concourse.bass as bass
    import concourse.mybir as mybir
    from concourse import bacc
    from concourse.tile import TileContext

    f32 = mybir.dt.float32
    f16 = mybir.dt.float16
    RELU = mybir.ActivationFunctionType.Relu

    nc = bacc.Bacc("TRN2", target_bir_lowering=False, debug=False,
                   num_devices=N_CORES)
    x_pp = nc.dram_tensor("x_pp", [128, 16 * 2 * BS], f16,
                          kind="ExternalInput")
    w1t = nc.dram_tensor("w1t", [128, 16 * 256], f16, kind="ExternalInput")
    w2p = nc.dram_tensor("w2p", [128, 21 * 1024], f16, kind="ExternalInput")
    w2s = nc.dram_tensor("w2s", [128, 7 * 512], f16, kind="ExternalInput")
    fc1m = nc.dram_tensor("fc1m", [128, 8 * 3200], f16, kind="ExternalInput")
    fc2t = nc.dram_tensor("fc2t", [128, 4 * 1024], f16, kind="ExternalInput")
    fc3t = nc.dram_tensor("fc3t", [128, 40], f16, kind="ExternalInput")
    y = nc.dram_tensor("y", [BS, 10], f32, kind="ExternalOutput")

    ectr = [0]

    with TileContext(nc) as tc:
        def relu_evac(dst, src):
            if ectr[0] % 2 == 0:
                nc.scalar.activation(dst, src, RELU)
            else:
                nc.vector.tensor_scalar_max(dst, src, 0.0)
            ectr[0] += 1

        with (
            tc.tile_pool(name="h2pool", bufs=25) as h2pool,
            tc.tile_pool(name="fcw", bufs=1) as fcw_pool,
            tc.tile_pool(name="scratch", bufs=1) as sc_pool,
        ):
            h2 = [h2pool.tile([128, 512], f16, tag="h2", name=f"h2_{T}")
                  for T in range(25)]
            ws = sc_pool.tile([128, 512], f16, tag="ws", name="ws")
            fc1wA = fcw_pool.tile([128, 4 * 3200], f16, tag="fc1wA",
                                  name="fc1wA")
            # --------------- phase 1: L1 + L2 interleaved ---------------
            with (
                tc.tile_pool(name="xp", bufs=1) as xpp_pool,
                tc.tile_pool(name="w1p", bufs=1) as w1_pool,
                tc.tile_pool(name="w2pp", bufs=3) as w2p_pool,
                tc.tile_pool(name="w2sp", bufs=1) as w2s_pool,
                tc.tile_pool(name="o1p", bufs=32) as o1_pool,
                tc.tile_pool(name="l1ps", bufs=2, space="PSUM") as l1ps,
                tc.tile_pool(name="l2ps", bufs=4, space="PSUM") as l2ps,
            ):
                # PE warmup with no DMA dependency: memset scratch, then
                # dummy MMs so HAM un-throttles while the first x chunk
                # streams in. K=128 full-row matmuls: HAM's activity
                # monitor does not register K=32 row-strip matmuls as
                # busy (observed: the clock stays at 4/8 until ~3.4us
                # after the first sustained full-row matmuls).
                nc.gpsimd.memset(ws[:], 0.0)
                nc.vector.memset(h2[24][64:128, :], 0.0)
                wps = l2ps.tile([128, 512], f32, tag="l2", name="warm_ps")

                def filler(n):
                    for wi in range(n):
                        nc.tensor.matmul(wps[:], ws[:, 0:128], ws[:],
                                         start=True, stop=True)

                # sized to bridge until the first x/w1/w2 chunks land
                # (~15us): fillers behind a stalled real matmul never
                # execute (in-order queue), so the bridge must be an
                # uninterrupted prefix.
                filler(24)

                # resident x + w1; w2 main pairs stream per-pass
                xall = xpp_pool.tile([128, 16 * 1024], f16, tag="xa",
                                     name="xall")
                w1a = w1_pool.tile([128, 16 * 256], f16, tag="w1",
                                   name="w1a")
                w2sa = w2s_pool.tile([128, 7 * 512], f16, tag="w2s",
                                     name="w2sa")
                w2c = []

                def dma_w2c(h):
                    t = w2p_pool.tile([128, 3072], f16, tag="w2c",
                                      name=f"w2c_{h}")
                    nc.sync.dma_start(out=t[:],
                                      in_=w2p.ap()[:, 3072*h:3072*h+3072])
                    w2c.append(t)

                def dma_x(c0, c1):
                    nc.sync.dma_start(out=xall[:, c0:c1],
                                      in_=x_pp.ap()[:, c0:c1])

                # issue order = HBM service order (HWDGE FIFO): match the
                # compute pipeline's consumption order.
                nc.sync.dma_start(out=w1a[:, 0:1536],   # rows 0-5
                                  in_=w1t.ap()[:, 0:1536])
                for i in range(4):   # rows 0-1 split by PE row-strip
                    nc.sync.dma_start(out=xall[32*i:32*i+32, 0:2048],
                                      in_=x_pp.ap()[32*i:32*i+32, 0:2048])
                dma_x(2048, 3072)    # row 2
                dma_x(3072, 4096)    # row 3
                dma_w2c(0)
                dma_x(4096, 5120)    # row 4
                dma_x(5120, 6144)    # row 5
                dma_w2c(1)
                dma_x(6144, 7168)    # row 6
                dma_x(7168, 8192)    # row 7
                nc.sync.dma_start(out=w1a[:, 1536:2560],  # rows 6-9
                                  in_=w1t.ap()[:, 1536:2560])
                nc.sync.dma_start(out=w2sa[:], in_=w2s.ap())
                dma_x(8192, 12288)   # rows 8-11
                dma_w2c(2)
                nc.sync.dma_start(out=w1a[:, 2560:4096],  # rows 10-15
                                  in_=w1t.ap()[:, 2560:4096])
                dma_w2c(3)
                dma_x(12288, 16384)  # rows 12-15
                dma_w2c(4)
                nc.sync.dma_start(out=fc1wA[:, 0:6400],
                                  in_=fc1m.ap()[:, 0:6400])
                dma_w2c(5)
                dma_w2c(6)
                nc.sync.dma_start(out=fc1wA[:, 6400:12800],
                                  in_=fc1m.ap()[:, 6400:12800])

                out1d = [[None] * 4 for _ in range(16)]

                def out1(r, p):
                    u = p % 2
                    d = p // 2
                    base = 512 * u
                    return out1d[r][d][:, base:base + 512]

                def xsl(r, g, i):
                    c0 = 1024 * r + 512 * g
                    return xall[32*i:32*i+32, c0:c0 + 512]

                def w1sl(r, g, i):
                    c0 = r * 256 + g * 128
                    return w1a[32*i:32*i+32, c0:c0 + 128]

                def emit_l1_group(r, g):
                    pss = []
                    for d in range(2):
                        ps = l1ps.tile([128, 1024], f32, tag="l1",
                                       name=f"l1ps_{r}_{g}_{d}")
                        pss.append(ps)
                        for u in range(2):
                            i = 2 * d + u
                            nc.tensor.matmul(
                                ps[:, 512*u:512*u+512],
                                w1sl(r, g, i), xsl(r, g, i),
                                start=True, stop=True,
                                tile_position=(32 * i, 0))
                    for d in range(2):
                        ot = o1_pool.tile([128, 1024], f16, tag="o1",
                                          name=f"o1_{r}_{g}_{d}")
                        relu_evac(ot[:], pss[d][:])
                        out1d[r][2 * g + d] = ot

                def emit_main_pair(h, j):
                    T = 3 * h + j
                    wt = w2c[h][:, 1024*j:1024*j+1024]
                    ps = l2ps.tile([128, 512], f32, tag="l2",
                                   name=f"l2ps_{T}")
                    # combined full-width chain on the shared middle pair
                    for kh in range(4):
                        nc.tensor.matmul(
                            ps[:], wt[:, 128*kh:128*kh+128],
                            out1(2*h + kh, 2*j + 1),
                            start=(kh == 0), stop=False,
                            tile_position=(0, 0))
                    # outer pairs: A (left) on cols 0-63, B (right) on
                    # cols 64-127, interleaved for column-strip overlap
                    for kh in range(4):
                        nc.tensor.matmul(
                            ps[0:64, :], wt[:, 512+64*kh:512+64*kh+64],
                            out1(2*h + kh, 2*j),
                            start=False, stop=(kh == 3),
                            tile_position=(0, 0))
                        nc.tensor.matmul(
                            ps[64:128, :], wt[:, 768+64*kh:768+64*kh+64],
                            out1(2*h + kh, 2*j + 2),
                            start=False, stop=(kh == 3),
                            tile_position=(0, 64))
                    relu_evac(h2[T][:], ps[:])

                def emit_cross_pair(pi):
                    # positions (2pi, 6) and (2pi+1, 6) on column strips,
                    # separate PSUM banks (no shared rhs tile)
                    T = 21 + pi
                    hA, hB = 2 * pi, 2 * pi + 1
                    wA = w2sa[:, 512 * hA:512 * hA + 512]
                    wB = w2sa[:, 512 * hB:512 * hB + 512] if hB < 7 else None
                    psA = l2ps.tile([128, 512], f32, tag="l2",
                                    name=f"l2psA_{T}")
                    psB = None
                    if wB is not None:
                        psB = l2ps.tile([128, 512], f32, tag="l2",
                                        name=f"l2psB_{T}")
                    for kt in range(8):
                        kh, t = divmod(kt, 2)
                        nc.tensor.matmul(
                            psA[0:64, :], wA[:, 64*kt:64*kt+64],
                            out1(2*hA + kh, 6 + t),
                            start=(kt == 0), stop=(kt == 7),
                            tile_position=(0, 0))
                        if wB is not None:
                            nc.tensor.matmul(
                                psB[64:128, :], wB[:, 64*kt:64*kt+64],
                                out1(2*hB + kh, 6 + t),
                                start=(kt == 0), stop=(kt == 7),
                                tile_position=(0, 64))
                    relu_evac(h2[T][0:64, :], psA[0:64, :])
                    if wB is not None:
                        relu_evac(h2[T][64:128, :], psB[64:128, :])

                # lead-in: rows 0-3 (pass 0 depends on them); fillers
                # between groups keep the PE free of idle windows long
                # enough for HAM to re-throttle while the first DMAs land
                for r in range(4):
                    for g in range(2):
                        emit_l1_group(r, g)
                        filler(2)
                # FC1 m-block 0 runs interleaved into late phase 1: its
                # weights (fc1wA) land ~40us and its early-k h2 inputs are
                # produced by passes 0-4, so its matmuls soak up residual
                # phase-1 stalls and shorten phase 2 by one chain.
                m0ps = [None]
                m0_first = [True]

                def emit_m0_seg(ks, last=False):
                    if m0ps[0] is None:
                        m0ps[0] = l2ps.tile([128, 512], f32, tag="l2",
                                            name="m0ps")
                    for n, k in enumerate(ks):
                        nc.tensor.matmul(
                            m0ps[0][:], fc1wA[:, 128*k:128*k+128],
                            h2[k][:],
                            start=m0_first[0],
                            stop=(last and n == len(ks) - 1))
                        m0_first[0] = False

                M0SEGS = {3: list(range(9)) + [21], 4: [9, 10, 11, 22],
                          5: [12, 13, 14]}
                # steady state: interleave pass-h L2 chains with the L1
                # groups of rows 2h+4/2h+5 so every l1ps double's evac
                # hides under an L2 chain (l1ps has only 2 slots)
                for h in range(6):
                    chains = [lambda j=j: emit_main_pair(h, j)
                              for j in range(3)]
                    if h in (1, 3, 5):
                        chains.append(lambda: emit_cross_pair((h - 1) // 2))
                    groups = [(2*h + 4, 0), (2*h + 4, 1),
                              (2*h + 5, 0), (2*h + 5, 1)]
                    for k in range(4):
                        if k < len(chains):
                            chains[k]()
                        emit_l1_group(*groups[k])
                    if h in M0SEGS:
                        emit_m0_seg(M0SEGS[h])
                for j in range(3):
                    emit_main_pair(6, j)
                emit_cross_pair(3)
                emit_m0_seg([15, 16, 17, 18, 19, 20, 23, 24], last=True)
                h3_0 = fcw_pool.tile([128, 512], f16, tag="h30",
                                     name="h3_0")
                relu_evac(h3_0[:], m0ps[0][:])

            # --------------- phase 2: FC head ---------------
            with (
                tc.tile_pool(name="fcio", bufs=12) as fcio_pool,
                tc.tile_pool(name="fcw2", bufs=1) as fcw2_pool,
                tc.tile_pool(name="fcps", bufs=2, space="PSUM") as fcps,
                tc.tile_pool(name="fc3ps", bufs=2, space="PSUM") as fc3ps,
            ):
                # second fc1 half + fc2/fc3 load into SBUF space freed by
                # the phase-1 pools, hidden under FC1 m0-m3 matmuls
                fc1wB = fcw2_pool.tile([128, 4 * 3200], f16, tag="fc1wB",
                                       name="fc1wB")
                nc.sync.dma_start(out=fc1wB[:, 0:6400],
                                  in_=fc1m.ap()[:, 12800:19200])
                nc.sync.dma_start(out=fc1wB[:, 6400:12800],
                                  in_=fc1m.ap()[:, 19200:25600])
                fc2w = fcw2_pool.tile([128, 4 * 1024], f16, tag="fc2w",
                                      name="fc2w")
                nc.sync.dma_start(out=fc2w[:], in_=fc2t.ap())
                w3 = fcw2_pool.tile([128, 40], f16, tag="fc3w", name="fc3w")
                nc.sync.dma_start(out=w3[:], in_=fc3t.ap())
                h3 = [h3_0]
                for m in range(1, 8):
                    wsrc = (fc1wA if m < 4 else fc1wB)
                    mo = 3200 * (m % 4)
                    ps = fcps.tile([128, 512], f32, tag="fc",
                                   name=f"fc1ps_{m}")
                    for k in range(25):
                        nc.tensor.matmul(
                            ps[:], wsrc[:, mo+128*k:mo+128*k+128],
                            h2[k][:],
                            start=(k == 0), stop=(k == 24))
                    ot = fcio_pool.tile([128, 512], f16, tag="h3",
                                        name=f"h3_{m}", bufs=8)
                    relu_evac(ot[:], ps[:])
                    h3.append(ot)
                h4 = []
                for m in range(4):
                    ps = fcps.tile([128, 512], f32, tag="fc",
                                   name=f"fc2ps_{m}")
                    for k in range(8):
                        nc.tensor.matmul(
                            ps[:],
                            fc2w[:, 1024*m+128*k:1024*m+128*k+128],
                            h3[k][:],
                            start=(k == 0), stop=(k == 7))
                    ot = fcio_pool.tile([128, 512], f16, tag="h4",
                                        name=f"h4_{m}", bufs=4)
                    relu_evac(ot[:], ps[:])
                    h4.append(ot)
                yt = fcio_pool.tile([128, 40], f32, tag="yout",
                                    name="yt", bufs=1)
                for b4 in range(4):
                    ps = fc3ps.tile([128, 10], f32, tag="fc3",
                                    name=f"fc3ps_{b4}")
                    for k in range(4):
                        nc.tensor.matmul(
                            ps[:],
                            h4[k][:, 128*b4:128*b4+128],
                            w3[:, 10*k:10*k+10],
                            start=(k == 0), stop=(k == 3))
                    nc.vector.tensor_copy(yt[:, 10*b4:10*b4+10], ps[:])
                nc.sync.dma_start(
                    out=y.ap().rearrange("(c p) o -> p c o", c=4),
                    in_=yt[:].rearrange("p (c o) -> p c o", c=4))
    nc.compile()
    return nc


def kernel(x, conv1w, conv2w, fc1, fc2, fc3):
    global LAST_EXEC_NS
    from concourse.bass_utils import run_bass_kernel_spmd

    x = np.ascontiguousarray(np.asarray(x, dtype=np.float32))
    conv1w = np.ascontiguousarray(np.asarray(conv1w, dtype=np.float32))
    conv2w = np.ascontiguousarray(np.asarray(conv2w, dtype=np.float32))
    fc1 = np.ascontiguousarray(np.asarray(fc1, dtype=np.float32))
    fc2 = np.ascontiguousarray(np.asarray(fc2, dtype=np.float32))
    fc3 = np.ascontiguousarray(np.asarray(fc3, dtype=np.float32))

    if not _NC_CACHE:
        _NC_CACHE.append(_build_nc())
    nc = _NC_CACHE[0]

    xpp = _prep_x(x)
    w2pm, w2sm = _prep_w2(conv2w)
    shared = {
        "w1t": _prep_w1(conv1w),
        "w2p": w2pm,
        "w2s": w2sm,
        "fc1m": _prep_fc1(fc1),
        "fc2t": _prep_fc2(fc2),
        "fc3t": _prep_fc3(fc3),
    }
    in_maps = [{**shared, "x_pp": xpp[c]} for c in range(N_CORES)]
    res = run_bass_kernel_spmd(nc, in_maps, list(range(N_CORES)))
    LAST_EXEC_NS = res.exec_time_ns
    return np.concatenate([r["y"] for r in res.results], axis=0)
